# revision 1
# baseline (speedup 1.0000x reference)
"""DeltaNet forward kernel for 8 Trainium2 NeuronCores.

Problem (hardcoded from the task spec): hidden_states [B=4, T=2048, D=1024],
H=4 heads, Dh=256, causal depthwise conv K=4 + silu on q/k/v projections,
q/k l2-normalized per head (q scaled Dh^-0.5), delta-rule recurrence over T,
per-head RMSNorm, merge heads, out = o @ Wo.

Sharding: data-parallel over (batch, head-group): core c -> batch c//2,
head group c%2 (projection columns [512*(c%2), 512*(c%2)+512)). Each core
computes a partial product against its 512 rows of Wo; the host sums the two
partials per batch (the unshard step for the row-parallel output matmul).

Device algorithm: chunked WY form of the delta rule, chunk C=128.
Per chunk (per head): KK = K K^T; A/B = strict lower/upper mask of KK;
R = (I+B)^{-1} = (I-B)(I+B^2)(I+B^4)...(I+B^64) via masked doubling
(B nilpotent); U' = R^T (V - K S); O = Q S + triu(K Q^T)^T U'; S += K^T U'.
S accumulates in PSUM in f32; matmul operands are bf16.
"""

import numpy as np

B, T, D = 4, 2048, 1024
H = 4
DH = D // H          # 256
CONV_K = 4
EPS = 1e-5
NCORES = 8
CG = 512             # columns per core (2 heads)
C = 128              # recurrence chunk length
NCHUNK = T // C      # 16
PAD = 4              # front zero padding on time axis for the causal conv
TOKB = 512           # token block (matmul moving size)
KT = D // 128        # 8 contraction tiles
CT = CG // 128       # 4 column tiles per core
NB = T // TOKB       # 4 token blocks

_CACHE = {}
SILU_NATIVE = True  # CoreSim lacks Silu; set False for simulation runs
DEBUG_SKIP_WO = False  # debug: skip output projection phase


def _build_bass():
    import concourse.bass as bass  # noqa: F401
    import concourse.bacc as bacc
    import concourse.mybir as mybir
    import concourse.tile as tile

    dt = mybir.dt
    nc = bacc.Bacc("TRN2", target_bir_lowering=False, debug=False)

    xT = nc.dram_tensor("xT", [D, T], dt.float16, kind="ExternalInput")
    wq = nc.dram_tensor("wq", [D, CG], dt.float16, kind="ExternalInput")
    wk = nc.dram_tensor("wk", [D, CG], dt.float16, kind="ExternalInput")
    wv = nc.dram_tensor("wv", [D, CG], dt.float16, kind="ExternalInput")
    wo = nc.dram_tensor("wo", [CG, D], dt.float16, kind="ExternalInput")
    cw = nc.dram_tensor("cw", [CG, 3 * CONV_K], dt.float32, kind="ExternalInput")
    consts = nc.dram_tensor("consts", [128, 6 * 128], dt.float16,
                            kind="ExternalInput")
    out = nc.dram_tensor("out", [T, D], dt.float32, kind="ExternalOutput")

    with tile.TileContext(nc) as tc:
        _body(nc, tc, mybir, xT, wq, wk, wv, wo, cw, consts, out)

    nc.compile()
    return nc


def _body(nc, tc, mybir, xT, wq, wk, wv, wo, cw, consts, out):
    dt = mybir.dt
    AF = mybir.ActivationFunctionType
    ALU = mybir.AluOpType
    fp32 = dt.float32
    bf16 = dt.float16  # 16-bit working dtype (fp16: 11-bit mantissa)
    NT = T + PAD

    xT_t = xT.ap().rearrange("(n p) t -> n p t", p=128)       # [8,128,T]
    w_t = {"q": wq.ap().rearrange("(n p) c -> n p c", p=128),
           "k": wk.ap().rearrange("(n p) c -> n p c", p=128),
           "v": wv.ap().rearrange("(n p) c -> n p c", p=128)}
    wo_t = wo.ap().rearrange("(n p) c -> n p c", p=128)       # [4,128,D]
    cw_t = cw.ap().rearrange("(n p) c -> n p c", p=128)       # [4,128,12]
    out_t = out.ap().rearrange("(n p) c -> n p c", p=128)     # [16,128,D]

    # ---------- persistent pool (lives for the whole kernel) ----------
    with tc.tile_pool(name="persist", bufs=1) as persist, \
         tc.tile_pool(name="qkvp", bufs=3 * CT) as qkvp, \
         tc.tile_pool(name="otp", bufs=CT) as otp, \
         tc.tile_pool(name="psw", bufs=6, space="PSUM") as psw, \
         tc.tile_pool(name="pss", bufs=1, space="PSUM") as pss:

        cons = persist.tile([128, 6 * 128], bf16, name="cons", tag="cons")
        nc.sync.dma_start(cons[:], consts.ap())
        ident = cons[:, 0:128]          # identity
        m_bdl = cons[:, 128:256]        # block-diag(32) strict lower, +1
        m_bdu = cons[:, 256:384]        # block-diag(32) strict upper, +1
        m_bdln = cons[:, 384:512]       # block-diag(32) strict lower, -1
        m_fneg = cons[:, 512:640]       # strict upper outside blocks, -1
        m_triuI = cons[:, 640:768]      # i<=j, +1
        ones_col = cons[:, 767:768]     # last col of (i<=j) mask == all ones

        biases = persist.tile([128, 3], dt.float32, name="biases", tag="biases")
        nc.vector.memset(biases[:, 0:1], 1e-6)
        nc.vector.memset(biases[:, 1:2], EPS)
        nc.vector.memset(biases[:, 2:3], 1e-6 * DH)

        cwt = []
        for ct in range(CT):
            t_ = persist.tile([128, 3 * CONV_K], fp32, name=f"cw{ct}",
                              tag=f"cw{ct}")
            nc.sync.dma_start(t_[:], cw_t[ct])
            cwt.append(t_)

        qh, kh, vh = [], [], []
        for lst, nm in ((qh, "q"), (kh, "k"), (vh, "v")):
            for ct in range(CT):
                lst.append(qkvp.tile([128, T], bf16, name=f"{nm}hat{ct}",
                                     tag="qkv"))
        oT = [otp.tile([128, T], bf16, name=f"oT{ct}", tag="oT")
              for ct in range(CT)]

        # ================= phase A: projections + conv + silu + l2norm ====
        with tc.tile_pool(name="xp", bufs=KT) as xp, \
             tc.tile_pool(name="wp", bufs=3 * KT) as wp, \
             tc.tile_pool(name="rawp", bufs=2) as rawp, \
             tc.tile_pool(name="sqp", bufs=4) as sqp, \
             tc.tile_pool(name="stp", bufs=1) as stp, \
             tc.tile_pool(name="bcp", bufs=2) as bcp:

            xt = []
            for kt in range(KT):
                t_ = xp.tile([128, T], bf16, name=f"xt{kt}", tag="xt")
                nc.sync.dma_start(t_[:], xT_t[kt])
                xt.append(t_)
            ws = {}
            for nm in ("q", "k", "v"):
                ws[nm] = []
                for kt in range(KT):
                    t_ = wp.tile([128, CG], bf16, name=f"w{nm}{kt}", tag="w")
                    nc.sync.dma_start(t_[:], w_t[nm][kt])
                    ws[nm].append(t_)

            for ti, (nm, dest) in enumerate((("q", qh), ("k", kh), ("v", vh))):
                sq_tiles = []
                for ct in range(CT):
                    rawt = rawp.tile([128, NT], bf16, name=f"raw{nm}{ct}",
                                     tag="raw")
                    nc.vector.memset(rawt[:, 0:PAD], 0.0)
                    dst = dest[ct]
                    for nb in range(NB):
                        pt = psw.tile([128, TOKB], fp32, name=f"pp{nm}{ct}{nb}",
                                      tag="w")
                        for kt in range(KT):
                            nc.tensor.matmul(
                                pt[:], ws[nm][kt][:, ct * 128:(ct + 1) * 128],
                                xt[kt][:, nb * TOKB:(nb + 1) * TOKB],
                                start=(kt == 0), stop=(kt == KT - 1))
                        nc.scalar.copy(
                            rawt[:, PAD + nb * TOKB:PAD + (nb + 1) * TOKB],
                            pt[:])
                    # causal depthwise conv along t
                    w0 = cwt[ct][:, ti * CONV_K:ti * CONV_K + 1]
                    nc.vector.tensor_scalar_mul(dst[:], rawt[:, 1:1 + T], w0)
                    for i in range(1, CONV_K):
                        wi = cwt[ct][:, ti * CONV_K + i:ti * CONV_K + i + 1]
                        nc.vector.scalar_tensor_tensor(
                            dst[:], rawt[:, 1 + i:1 + i + T], wi, dst[:],
                            ALU.mult, ALU.add)
                    if SILU_NATIVE:
                        nc.scalar.activation(dst[:], dst[:], AF.Silu)
                    else:
                        sg = rawp.tile([128, T], bf16, name=f"sg{nm}{ct}",
                                       tag="raw")
                        nc.scalar.activation(sg[:], dst[:], AF.Sigmoid)
                        nc.vector.tensor_mul(dst[:], dst[:], sg[:])
                    if ti < 2:
                        sqt = sqp.tile([128, T], bf16, name=f"sq{nm}{ct}",
                                       tag="sq")
                        nc.scalar.activation(sqt[:], dst[:], AF.Square)
                        sq_tiles.append(sqt)
                if ti < 2:
                    # per-head l2norm: sumsq rows via ones-matmul, broadcast
                    # to 128 partitions, rsq = scale/sqrt(ss + 1e-6), apply.
                    for head in range(2):
                        bcf = bcp.tile([128, T], fp32, name=f"bcf{nm}{head}",
                                       tag="bcf")
                        for nb in range(NB):
                            prow = psw.tile([1, TOKB], fp32,
                                            name=f"pr{nm}{head}{nb}", tag="w")
                            for cth in range(2):
                                nc.tensor.matmul(
                                    prow[:], ones_col,
                                    sq_tiles[head * 2 + cth][
                                        :, nb * TOKB:(nb + 1) * TOKB],
                                    start=(cth == 0), stop=(cth == 1))
                            rowb = stp.tile([1, TOKB], fp32,
                                            name=f"rb{nm}{head}{nb}",
                                            tag="rowb", bufs=3)
                            nc.scalar.copy(rowb[:], prow[:])
                            nc.gpsimd.partition_broadcast(
                                bcf[:, nb * TOKB:(nb + 1) * TOKB], rowb[:])
                        if ti == 0:
                            # fold Dh^-0.5: 1/(16 sqrt(ss+eps)) =
                            # 1/sqrt(256 ss + 256 eps)
                            nc.scalar.activation(bcf[:], bcf[:], AF.Sqrt,
                                                 bias=biases[:, 2:3],
                                                 scale=float(DH))
                        else:
                            nc.scalar.activation(bcf[:], bcf[:], AF.Sqrt,
                                                 bias=biases[:, 0:1])
                        nc.vector.reciprocal(bcf[:], bcf[:])
                        bcb = bcp.tile([128, T], bf16, name=f"bcb{nm}{head}",
                                       tag="bcb")
                        nc.scalar.copy(bcb[:], bcf[:])
                        for cth in range(2):
                            ct = head * 2 + cth
                            nc.vector.tensor_mul(dest[ct][:], dest[ct][:],
                                                 bcb[:])

        # ================= phase B: delta-rule recurrence =================
        with tc.tile_pool(name="recp", bufs=4) as recp, \
             tc.tile_pool(name="recs", bufs=1) as recs:
            for head in range(2):
                ct0 = head * 2
                s_ps = pss.tile([128, 512], fp32, name=f"sps{head}", tag="sps")
                s_sb = recs.tile([128, 512], bf16, name=f"ssb{head}", tag="ssb",
                                 bufs=2)
                nc.vector.memset(s_sb[:], 0.0)
                for ch in range(NCHUNK):
                    t0 = ch * C
                    QT = [qh[ct0][:, t0:t0 + C], qh[ct0 + 1][:, t0:t0 + C]]
                    KTt = [kh[ct0][:, t0:t0 + C], kh[ct0 + 1][:, t0:t0 + C]]
                    VT = [vh[ct0][:, t0:t0 + C], vh[ct0 + 1][:, t0:t0 + C]]

                    # K, V in [C, Dh] layout via PE transpose (bf16 psum)
                    kcd = recp.tile([128, 256], bf16, name=f"kcd{head}{ch}",
                                    tag="kcd")
                    vcd = recp.tile([128, 256], bf16, name=f"vcd{head}{ch}",
                                    tag="vcd")
                    for i in range(2):
                        ptk = psw.tile([128, 128], bf16, name=f"ptk{head}{ch}{i}",
                                       tag="w")
                        nc.tensor.transpose(ptk[:], KTt[i], ident)
                        nc.scalar.copy(kcd[:, i * 128:(i + 1) * 128], ptk[:])
                        ptv = psw.tile([128, 128], bf16, name=f"ptv{head}{ch}{i}",
                                       tag="w")
                        nc.tensor.transpose(ptv[:], VT[i], ident)
                        nc.scalar.copy(vcd[:, i * 128:(i + 1) * 128], ptv[:])

                    # KK^T; A, B, -A masks
                    pkk = psw.tile([128, 128], fp32, name=f"pkk{head}{ch}",
                                   tag="w")
                    for i in range(2):
                        nc.tensor.matmul(pkk[:], KTt[i], KTt[i], start=(i == 0),
                                         stop=(i == 1))
                    Nl = recp.tile([128, 128], bf16, name=f"Nl{head}{ch}",
                                   tag="Nl")
                    Nln = recp.tile([128, 128], bf16, name=f"Nln{head}{ch}",
                                    tag="Nln")
                    Nu = recp.tile([128, 128], bf16, name=f"Nu{head}{ch}",
                                   tag="Nu")
                    FnT = recp.tile([128, 128], bf16, name=f"FnT{head}{ch}",
                                    tag="FnT")
                    nc.vector.tensor_mul(Nl[:], pkk[:], m_bdl)
                    nc.vector.tensor_mul(Nu[:], pkk[:], m_bdu)
                    nc.vector.tensor_mul(Nln[:], pkk[:], m_bdln)
                    nc.vector.tensor_mul(FnT[:], pkk[:], m_fneg)

                    # R = D^T = (I+Nu)^{-1}, block-diag(32): 4 exact levels
                    pR = psw.tile([128, 128], fp32, name=f"pR{head}{ch}",
                                  tag="w")
                    nc.tensor.matmul(pR[:], ident, ident, start=True,
                                     stop=False)
                    nc.tensor.matmul(pR[:], Nln[:], ident, start=False,
                                     stop=True)
                    Rm = recp.tile([128, 128], bf16, name=f"Rm{head}{ch}0",
                                   tag="Rm")
                    nc.scalar.copy(Rm[:], pR[:])
                    Pm, Qm = Nl, Nu
                    for lvl in range(3):
                        pp = psw.tile([128, 128], fp32,
                                      name=f"pp{head}{ch}{lvl}", tag="w")
                        nc.tensor.matmul(pp[:], Qm[:], Pm[:], start=True,
                                         stop=True)
                        Pn = recp.tile([128, 128], bf16,
                                       name=f"Pn{head}{ch}{lvl}", tag="Pn")
                        nc.scalar.copy(Pn[:], pp[:])
                        if lvl < 2:
                            pq = psw.tile([128, 128], fp32,
                                          name=f"pq{head}{ch}{lvl}", tag="w")
                            nc.tensor.matmul(pq[:], Pm[:], Qm[:], start=True,
                                             stop=True)
                            Qn = recp.tile([128, 128], bf16,
                                           name=f"Qn{head}{ch}{lvl}", tag="Qn")
                            nc.scalar.copy(Qn[:], pq[:])
                        else:
                            Qn = None
                        nc.tensor.matmul(pR[:], Pn[:], Rm[:], start=False,
                                         stop=True,
                                         skip_group_check=True)
                        Rn = recp.tile([128, 128], bf16,
                                       name=f"Rm{head}{ch}{lvl + 1}", tag="Rm")
                        nc.scalar.copy(Rn[:], pR[:])
                        Rm, Pm, Qm = Rn, Pn, Qn

                    # RHS' = V - K S    (psum = K@S, then V - psum on DVE)
                    pks = psw.tile([128, 256], fp32, name=f"pks{head}{ch}",
                                   tag="w")
                    for i in range(2):
                        nc.tensor.matmul(pks[:], KTt[i],
                                         s_sb[:, i * 256:(i + 1) * 256],
                                         start=(i == 0), stop=(i == 1))
                    rhs_sb = recp.tile([128, 256], bf16, name=f"rhs{head}{ch}",
                                       tag="rhs")
                    nc.vector.tensor_sub(rhs_sb[:], vcd[:], pks[:])

                    # U' via block forward substitution (4 blocks of 32)
                    u_sb = recp.tile([128, 256], bf16, name=f"u{head}{ch}",
                                     tag="u")
                    y_sb = recp.tile([128, 256], bf16, name=f"y{head}{ch}",
                                     tag="y")
                    nc.vector.memset(u_sb[:], 0.0)
                    px = psw.tile([128, 256], fp32, name=f"px{head}{ch}",
                                  tag="w")
                    py = psw.tile([128, 256], fp32, name=f"py{head}{ch}",
                                  tag="w")
                    nc.tensor.matmul(px[0:32, :], Rm[0:32, 0:32],
                                     rhs_sb[0:32, :], start=True, stop=True,
                                     tile_position=(0, 0))
                    nc.vector.tensor_copy(u_sb[0:32, :], px[0:32, :])
                    for i in range(1, 4):
                        p0 = 32 * i
                        nc.tensor.matmul(py[p0:p0 + 32, :],
                                         FnT[:, p0:p0 + 32], u_sb[:],
                                         start=True, stop=True,
                                         tile_position=(0, p0))
                        nc.vector.tensor_add(y_sb[p0:p0 + 32, :],
                                             rhs_sb[p0:p0 + 32, :],
                                             py[p0:p0 + 32, :])
                        nc.tensor.matmul(px[p0:p0 + 32, :],
                                         Rm[p0:p0 + 32, p0:p0 + 32],
                                         y_sb[p0:p0 + 32, :],
                                         start=True, stop=True,
                                         tile_position=(p0, p0))
                        nc.vector.tensor_copy(u_sb[p0:p0 + 32, :],
                                              px[p0:p0 + 32, :])

                    # attn P = triu_incl(K Q^T)
                    pkq = psw.tile([128, 128], fp32, name=f"pkq{head}{ch}",
                                   tag="w")
                    for i in range(2):
                        nc.tensor.matmul(pkq[:], KTt[i], QT[i], start=(i == 0),
                                         stop=(i == 1))
                    Pat = recp.tile([128, 128], bf16, name=f"Pat{head}{ch}",
                                    tag="Pat")
                    nc.vector.tensor_mul(Pat[:], pkq[:], m_triuI)

                    # O = Q S + P^T U'
                    po = psw.tile([128, 256], fp32, name=f"po{head}{ch}",
                                  tag="w")
                    for i in range(2):
                        nc.tensor.matmul(po[:], QT[i],
                                         s_sb[:, i * 256:(i + 1) * 256],
                                         start=(i == 0), stop=False)
                    nc.tensor.matmul(po[:], Pat[:], u_sb[:], start=False,
                                     stop=True)

                    # S += K^T U'   (accumulate in persistent psum)
                    for i in range(2):
                        nc.tensor.matmul(s_ps[:, i * 256:(i + 1) * 256],
                                         kcd[:, i * 128:(i + 1) * 128], u_sb[:],
                                         start=(ch == 0 and i == 0), stop=True,
                                         skip_group_check=True)
                    s_nb = recs.tile([128, 512], bf16, name=f"ssb{head}{ch}",
                                     tag="ssb", bufs=2)
                    nc.vector.tensor_copy(s_nb[:], s_ps[:])
                    s_sb = s_nb

                    # RMSNorm rows of O, then transpose out to oT
                    osq = recp.tile([128, 256], bf16, name=f"osq{head}{ch}",
                                    tag="osq")
                    ossq = recp.tile([128, 1], fp32, name=f"ossq{head}{ch}",
                                     tag="ossq")
                    nc.scalar.activation(osq[:], po[:], AF.Square,
                                         accum_out=ossq[:])
                    orsq = recp.tile([128, 1], fp32, name=f"orsq{head}{ch}",
                                     tag="orsq")
                    nc.scalar.activation(orsq[:], ossq[:], AF.Sqrt,
                                         bias=biases[:, 1:2], scale=1.0 / DH)
                    nc.vector.reciprocal(orsq[:], orsq[:])
                    onrm = recp.tile([128, 256], bf16, name=f"onrm{head}{ch}",
                                     tag="onrm")
                    nc.vector.tensor_scalar_mul(onrm[:], po[:], orsq[:])
                    for i in range(2):
                        pto = psw.tile([128, 128], bf16,
                                       name=f"pto{head}{ch}{i}", tag="w")
                        nc.tensor.transpose(pto[:], onrm[:, i * 128:(i + 1) * 128],
                                            ident)
                        nc.scalar.copy(oT[ct0 + i][:, t0:t0 + C], pto[:])

        # ================= phase C: output projection =====================
        if DEBUG_SKIP_WO:
            return
        with tc.tile_pool(name="wop", bufs=CT) as wop, \
             tc.tile_pool(name="ofp", bufs=3) as ofp:
            wo_s = []
            for ct in range(CT):
                t_ = wop.tile([128, D], bf16, name=f"wo{ct}", tag="wo")
                nc.sync.dma_start(t_[:], wo_t[ct])
                wo_s.append(t_)
            for tt in range(T // 128):
                for half in range(2):
                    pf = psw.tile([128, 512], fp32, name=f"pf{tt}{half}",
                                  tag="w")
                    for ct in range(CT):
                        nc.tensor.matmul(
                            pf[:], oT[ct][:, tt * 128:(tt + 1) * 128],
                            wo_s[ct][:, half * 512:(half + 1) * 512],
                            start=(ct == 0), stop=(ct == CT - 1))
                    of = ofp.tile([128, 512], fp32, name=f"of{tt}{half}",
                                  tag="of")
                    nc.scalar.copy(of[:], pf[:])
                    nc.sync.dma_start(
                        out_t[tt][:, half * 512:(half + 1) * 512], of[:])


LP_NP = np.float16  # host-side 16-bit dtype matching the device dtype


def _make_consts():
    ii = np.arange(128)
    blk = ii[:, None] // 32 == ii[None, :] // 32
    ident = np.eye(128, dtype=np.float32)
    bdl = ((ii[:, None] > ii[None, :]) & blk).astype(np.float32)
    bdu = ((ii[:, None] < ii[None, :]) & blk).astype(np.float32)
    fneg = -((ii[:, None] < ii[None, :]) & ~blk).astype(np.float32)
    triuI = (ii[:, None] <= ii[None, :]).astype(np.float32)
    return np.concatenate([ident, bdl, bdu, -bdl, fneg, triuI],
                          axis=1).astype(LP_NP)


def _get_compiled():
    key = ("nc", SILU_NATIVE)
    if key not in _CACHE:
        _CACHE[key] = _build_bass()
    return _CACHE[key]


def kernel(hidden_states, Wq, Wk, Wv, conv_wq, conv_wk, conv_wv, onorm_w, Wo):
    from concourse.bass_utils import run_bass_kernel_spmd

    hidden_states = np.asarray(hidden_states, np.float32)
    Wq = np.asarray(Wq, np.float32)
    Wk = np.asarray(Wk, np.float32)
    Wv = np.asarray(Wv, np.float32)
    Wo = np.asarray(Wo, np.float32)
    conv_wq = np.asarray(conv_wq, np.float32)
    conv_wk = np.asarray(conv_wk, np.float32)
    conv_wv = np.asarray(conv_wv, np.float32)
    onorm_w = np.asarray(onorm_w, np.float32)

    bf = LP_NP
    consts = _make_consts()
    Wo_eff = (Wo * np.tile(onorm_w, H)[:, None]).astype(bf)  # fold RMS weight

    in_maps = []
    for core in range(NCORES):
        b, g = divmod(core, 2)
        cols = slice(CG * g, CG * (g + 1))
        in_maps.append({
            "xT": np.ascontiguousarray(hidden_states[b].T).astype(bf),
            "wq": np.ascontiguousarray(Wq[:, cols]).astype(bf),
            "wk": np.ascontiguousarray(Wk[:, cols]).astype(bf),
            "wv": np.ascontiguousarray(Wv[:, cols]).astype(bf),
            "wo": np.ascontiguousarray(Wo_eff[cols, :]),
            "cw": np.ascontiguousarray(np.concatenate(
                [conv_wq[cols], conv_wk[cols], conv_wv[cols]], axis=1)),
            "consts": consts,
        })

    nc = _get_compiled()
    res = run_bass_kernel_spmd(nc, in_maps, core_ids=list(range(NCORES)),
                               **_CACHE.get("run_kwargs", {}))
    _CACHE["last_results"] = res
    out = np.zeros((B, T, D), np.float32)
    for core in range(NCORES):
        out[core // 2] += res.results[core]["out"]
    return out



# revision 7
# speedup vs baseline: 1.2205x; 1.2205x over previous
"""DeltaNet forward kernel for 8 Trainium2 NeuronCores.

Problem (hardcoded from the task spec): hidden_states [B=4, T=2048, D=1024],
H=4 heads, Dh=256, causal depthwise conv K=4 + silu on q/k/v projections,
q/k l2-normalized per head (q scaled Dh^-0.5), delta-rule recurrence over T,
per-head RMSNorm, merge heads, out = o @ Wo.

Sharding: data-parallel over (batch, head-group): core c -> batch c//2,
head group c%2 (projection columns [512*(c%2), 512*(c%2)+512)). Each core
computes a partial product against its 512 rows of Wo; the host sums the two
partials per batch (the unshard step for the row-parallel output matmul).

Device algorithm (decoupled WY form, chunk C=128): per chunk and head all
S-independent matrices are precomputed off the critical path:
  KK = K K^T;  R_bd^T = (I+B)^{-1} (B = strict upper of KK in 64-blocks)
  via a power ladder exact to B^15;  R^T = R_bd^T (I - G^T) with
  G = R_bd F (2 blocks of 64 -> G^2 = 0, exact);  M^T = K^T R^T.
The sequential S-chain is only:  U' = R V - M S_prev  (psum-accumulated),
S += K^T U', plus two psum->sbuf copies. O = Q S_prev + triu(K Q^T)^T U',
then per-head RMSNorm and transpose into the output-projection layout.
The Wo projection for token tile tt is emitted right after chunk tt so it
overlaps the recurrence. S accumulates in PSUM f32; matmul operands fp16.
"""

import numpy as np

B, T, D = 4, 2048, 1024
H = 4
DH = D // H          # 256
CONV_K = 4
EPS = 1e-5
NCORES = 8
CG = 512             # columns per core (2 heads)
C = 128              # recurrence chunk length
NCHUNK = T // C      # 16
PAD = 4              # front zero padding on time axis for the causal conv
TOKB = 512           # token block (matmul moving size)
KT = D // 128        # 8 contraction tiles
CT = CG // 128       # 4 column tiles per core
NB = T // TOKB       # 4 token blocks

_CACHE = {}
SILU_NATIVE = True  # CoreSim lacks Silu; set False for simulation runs


def _build_bass():
    import concourse.bass as bass  # noqa: F401
    import concourse.bacc as bacc
    import concourse.mybir as mybir
    import concourse.tile as tile

    dt = mybir.dt
    nc = bacc.Bacc("TRN2", target_bir_lowering=False, debug=False)

    xT = nc.dram_tensor("xT", [D, T], dt.float16, kind="ExternalInput")
    wq = nc.dram_tensor("wq", [D, CG], dt.float16, kind="ExternalInput")
    wk = nc.dram_tensor("wk", [D, CG], dt.float16, kind="ExternalInput")
    wv = nc.dram_tensor("wv", [D, CG], dt.float16, kind="ExternalInput")
    wo = nc.dram_tensor("wo", [CG, D], dt.float16, kind="ExternalInput")
    cw = nc.dram_tensor("cw", [CG, 3 * CONV_K], dt.float32, kind="ExternalInput")
    consts = nc.dram_tensor("consts", [128, 5 * 128], dt.float16,
                            kind="ExternalInput")
    out = nc.dram_tensor("out", [T, D], dt.float32, kind="ExternalOutput")

    with tile.TileContext(nc) as tc:
        _body(nc, tc, mybir, xT, wq, wk, wv, wo, cw, consts, out)

    nc.compile()
    return nc


def _body(nc, tc, mybir, xT, wq, wk, wv, wo, cw, consts, out):
    dt = mybir.dt
    AF = mybir.ActivationFunctionType
    ALU = mybir.AluOpType
    fp32 = dt.float32
    bf16 = dt.float16  # 16-bit working dtype (fp16: 11-bit mantissa)
    NT = T + PAD

    xT_t = xT.ap().rearrange("(n p) t -> n p t", p=128)       # [8,128,T]
    w_t = {"q": wq.ap().rearrange("(n p) c -> n p c", p=128),
           "k": wk.ap().rearrange("(n p) c -> n p c", p=128),
           "v": wv.ap().rearrange("(n p) c -> n p c", p=128)}
    wo_t = wo.ap().rearrange("(n p) c -> n p c", p=128)       # [4,128,D]
    cw_t = cw.ap().rearrange("(n p) c -> n p c", p=128)       # [4,128,12]
    out_t = out.ap().rearrange("(n p) c -> n p c", p=128)     # [16,128,D]

    # ---------- persistent pool (lives for the whole kernel) ----------
    with tc.tile_pool(name="persist", bufs=1) as persist, \
         tc.tile_pool(name="qkvp", bufs=3 * CT) as qkvp, \
         tc.tile_pool(name="otp", bufs=6) as otp, \
         tc.tile_pool(name="wop", bufs=CT) as wop, \
         tc.tile_pool(name="psw", bufs=6, space="PSUM") as psw, \
         tc.tile_pool(name="pss", bufs=1, space="PSUM") as pss:

        cons = persist.tile([128, 5 * 128], bf16, name="cons", tag="cons")
        nc.sync.dma_start(cons[:], consts.ap())
        ident = cons[:, 0:128]          # identity
        m_nln = cons[:, 128:256]        # -(strict lower, in 64-block)
        m_nun = cons[:, 256:384]        # -(strict upper, in 64-block)
        m_fln = cons[:, 384:512]        # -(strict lower, outside 64-blocks)
        m_triuI = cons[:, 512:640]      # i<=j, +1
        ones_col = cons[:, 639:640]     # last col of (i<=j) mask == all ones

        biases = persist.tile([128, 3], dt.float32, name="biases", tag="biases")
        nc.vector.memset(biases[:, 0:1], 1e-6)
        nc.vector.memset(biases[:, 1:2], EPS)
        nc.vector.memset(biases[:, 2:3], 1e-6 * DH)

        wo_s = []
        for ct in range(CT):
            t_ = wop.tile([128, D], bf16, name=f"wo{ct}", tag="wo")
            nc.sync.dma_start(t_[:], wo_t[ct])
            wo_s.append(t_)

        cwt = []
        for ct in range(CT):
            t_ = persist.tile([128, 3 * CONV_K], fp32, name=f"cw{ct}",
                              tag=f"cw{ct}")
            nc.sync.dma_start(t_[:], cw_t[ct])
            cwt.append(t_)

        qh, kh, vh = [], [], []
        for lst, nm in ((qh, "q"), (kh, "k"), (vh, "v")):
            for ct in range(CT):
                lst.append(qkvp.tile([128, T], bf16, name=f"{nm}hat{ct}",
                                     tag="qkv"))

        # ================= phase A: projections + conv + silu + l2norm ====
        with tc.tile_pool(name="xp", bufs=KT) as xp, \
             tc.tile_pool(name="wp", bufs=3 * KT) as wp, \
             tc.tile_pool(name="rawp", bufs=2) as rawp, \
             tc.tile_pool(name="sqp", bufs=4) as sqp, \
             tc.tile_pool(name="stp", bufs=1) as stp, \
             tc.tile_pool(name="bcp", bufs=2) as bcp:

            xt = []
            for kt in range(KT):
                t_ = xp.tile([128, T], bf16, name=f"xt{kt}", tag="xt")
                nc.sync.dma_start(t_[:], xT_t[kt])
                xt.append(t_)
            ws = {}
            for nm in ("q", "k", "v"):
                ws[nm] = []
                for kt in range(KT):
                    t_ = wp.tile([128, CG], bf16, name=f"w{nm}{kt}", tag="w")
                    nc.sync.dma_start(t_[:], w_t[nm][kt])
                    ws[nm].append(t_)

            for ti, (nm, dest) in enumerate((("q", qh), ("k", kh), ("v", vh))):
                sq_tiles = []
                for ct in range(CT):
                    rawt = rawp.tile([128, NT], bf16, name=f"raw{nm}{ct}",
                                     tag="raw")
                    nc.vector.memset(rawt[:, 0:PAD], 0.0)
                    dst = dest[ct]
                    for nb in range(NB):
                        pt = psw.tile([128, TOKB], fp32, name=f"pp{nm}{ct}{nb}",
                                      tag="w")
                        for kt in range(KT):
                            nc.tensor.matmul(
                                pt[:], ws[nm][kt][:, ct * 128:(ct + 1) * 128],
                                xt[kt][:, nb * TOKB:(nb + 1) * TOKB],
                                start=(kt == 0), stop=(kt == KT - 1))
                        nc.scalar.copy(
                            rawt[:, PAD + nb * TOKB:PAD + (nb + 1) * TOKB],
                            pt[:])
                    # causal depthwise conv along t
                    w0 = cwt[ct][:, ti * CONV_K:ti * CONV_K + 1]
                    nc.vector.tensor_scalar_mul(dst[:], rawt[:, 1:1 + T], w0)
                    for i in range(1, CONV_K):
                        wi = cwt[ct][:, ti * CONV_K + i:ti * CONV_K + i + 1]
                        nc.vector.scalar_tensor_tensor(
                            dst[:], rawt[:, 1 + i:1 + i + T], wi, dst[:],
                            ALU.mult, ALU.add)
                    if SILU_NATIVE:
                        nc.scalar.activation(dst[:], dst[:], AF.Silu)
                    else:
                        sg = rawp.tile([128, T], bf16, name=f"sg{nm}{ct}",
                                       tag="raw")
                        nc.scalar.activation(sg[:], dst[:], AF.Sigmoid)
                        nc.vector.tensor_mul(dst[:], dst[:], sg[:])
                    if ti < 2:
                        sqt = sqp.tile([128, T], bf16, name=f"sq{nm}{ct}",
                                       tag="sq")
                        nc.scalar.activation(sqt[:], dst[:], AF.Square)
                        sq_tiles.append(sqt)
                if ti < 2:
                    # per-head l2norm: sumsq rows via ones-matmul, broadcast
                    # to 128 partitions, rsq = scale/sqrt(ss + 1e-6), apply.
                    for head in range(2):
                        bcf = bcp.tile([128, T], fp32, name=f"bcf{nm}{head}",
                                       tag="bcf")
                        for nb in range(NB):
                            prow = psw.tile([1, TOKB], fp32,
                                            name=f"pr{nm}{head}{nb}", tag="w")
                            for cth in range(2):
                                nc.tensor.matmul(
                                    prow[:], ones_col,
                                    sq_tiles[head * 2 + cth][
                                        :, nb * TOKB:(nb + 1) * TOKB],
                                    start=(cth == 0), stop=(cth == 1))
                            rowb = stp.tile([1, TOKB], fp32,
                                            name=f"rb{nm}{head}{nb}",
                                            tag="rowb", bufs=3)
                            nc.scalar.copy(rowb[:], prow[:])
                            nc.gpsimd.partition_broadcast(
                                bcf[:, nb * TOKB:(nb + 1) * TOKB], rowb[:])
                        if ti == 0:
                            # fold Dh^-0.5: 1/(16 sqrt(ss+eps)) =
                            # 1/sqrt(256 ss + 256 eps)
                            nc.scalar.activation(bcf[:], bcf[:], AF.Sqrt,
                                                 bias=biases[:, 2:3],
                                                 scale=float(DH))
                        else:
                            nc.scalar.activation(bcf[:], bcf[:], AF.Sqrt,
                                                 bias=biases[:, 0:1])
                        nc.vector.reciprocal(bcf[:], bcf[:])
                        bcb = bcp.tile([128, T], bf16, name=f"bcb{nm}{head}",
                                       tag="bcb")
                        nc.scalar.copy(bcb[:], bcf[:])
                        for cth in range(2):
                            ct = head * 2 + cth
                            nc.vector.tensor_mul(dest[ct][:], dest[ct][:],
                                                 bcb[:])

        # ================= phase B: delta-rule recurrence + phase C =======
        with tc.tile_pool(name="recp", bufs=3) as recp, \
             tc.tile_pool(name="recs", bufs=1) as recs, \
             tc.tile_pool(name="ofp", bufs=3) as ofp:
            s_ps, s_sb = {}, {}
            for head in range(2):
                s_ps[head] = pss.tile([128, 512], fp32, name=f"sps{head}",
                                      tag=f"sps{head}")
                s_sb[head] = recs.tile([128, 512], bf16, name=f"ssb{head}i",
                                       tag=f"ssb{head}", bufs=2)
                nc.vector.memset(s_sb[head][:], 0.0)

            for ch in range(NCHUNK):
                t0 = ch * C
                oTt = otp.tile([128, 512], bf16, name=f"oT{ch}", tag="oT")
                for head in range(2):
                    ct0 = head * 2
                    QT = [qh[ct0][:, t0:t0 + C], qh[ct0 + 1][:, t0:t0 + C]]
                    KTt = [kh[ct0][:, t0:t0 + C], kh[ct0 + 1][:, t0:t0 + C]]
                    VT = [vh[ct0][:, t0:t0 + C], vh[ct0 + 1][:, t0:t0 + C]]
                    hc = f"{head}_{ch}"

                    # K, V in [C, Dh] via PE transpose into one psum bank
                    pkv = psw.tile([128, 512], bf16, name=f"pkv{hc}", tag="w")
                    for i in range(2):
                        nc.tensor.transpose(pkv[:, i * 128:(i + 1) * 128],
                                            KTt[i], ident)
                        nc.tensor.transpose(pkv[:, 256 + i * 128:384 + i * 128],
                                            VT[i], ident)
                    kvcd = recp.tile([128, 512], bf16, name=f"kvcd{hc}",
                                     tag="kvcd")
                    nc.scalar.copy(kvcd[:], pkv[:])
                    kcd = kvcd[:, 0:256]
                    vcd = kvcd[:, 256:512]

                    # KK^T and masked pieces
                    pkk = psw.tile([128, 128], fp32, name=f"pkk{hc}", tag="w")
                    for i in range(2):
                        nc.tensor.matmul(pkk[:], KTt[i], KTt[i], start=(i == 0),
                                         stop=(i == 1))
                    mNl = recp.tile([128, 128], bf16, name=f"mNl{hc}",
                                    tag="mNl")
                    mNu = recp.tile([128, 128], bf16, name=f"mNu{hc}",
                                    tag="mNu")
                    mFl = recp.tile([128, 128], bf16, name=f"mFl{hc}",
                                    tag="mFl")
                    nc.vector.tensor_mul(mNl[:], pkk[:], m_nln)    # -B^T
                    nc.vector.tensor_mul(mNu[:], pkk[:], m_nun)    # -B
                    nc.vector.tensor_mul(mFl[:], pkk[:], m_fln)    # -F

                    # ladder: Rm = sum_{j<=15} (-B)^j  (B strict upper in-64)
                    pl = psw.tile([128, 256], fp32, name=f"pl{hc}", tag="w")
                    nc.tensor.matmul(pl[:, 0:128], mNl[:], mNu[:],
                                     start=True, stop=True)        # B^2
                    nc.tensor.matmul(pl[:, 128:256], mNu[:], mNl[:],
                                     start=True, stop=True)        # Nl^2
                    sb2 = recp.tile([128, 256], bf16, name=f"sb2{hc}",
                                    tag="sb2")
                    nc.scalar.copy(sb2[:], pl[:])
                    b2, b2l = sb2[:, 0:128], sb2[:, 128:256]

                    pt1 = psw.tile([128, 128], fp32, name=f"pt1{hc}", tag="w")
                    nc.tensor.matmul(pt1[:], ident, ident, start=True,
                                     stop=False)
                    nc.tensor.matmul(pt1[:], ident, mNu[:], start=False,
                                     stop=False)
                    nc.tensor.matmul(pt1[:], ident, b2, start=False,
                                     stop=False)
                    nc.tensor.matmul(pt1[:], mNl[:], b2, start=False,
                                     stop=True)                    # -B^3
                    sT1 = recp.tile([128, 128], bf16, name=f"sT1{hc}",
                                    tag="sT1")
                    nc.scalar.copy(sT1[:], pt1[:])

                    pl2 = psw.tile([128, 256], fp32, name=f"pl2{hc}", tag="w")
                    nc.tensor.matmul(pl2[:, 0:128], b2, b2l,
                                     start=True, stop=True)        # Nl^4
                    nc.tensor.matmul(pl2[:, 128:256], b2l, b2,
                                     start=True, stop=True)        # Nu^4
                    sb4 = recp.tile([128, 256], bf16, name=f"sb4{hc}",
                                    tag="sb4")
                    nc.scalar.copy(sb4[:], pl2[:])
                    b4l, b4u = sb4[:, 0:128], sb4[:, 128:256]

                    pr8 = psw.tile([128, 256], fp32, name=f"pr8{hc}", tag="w")
                    nc.tensor.matmul(pr8[:, 0:128], ident, sT1[:],
                                     start=True, stop=False)
                    nc.tensor.matmul(pr8[:, 0:128], b4l, sT1[:],
                                     start=False, stop=True)       # (I+B^4)T1
                    nc.tensor.matmul(pr8[:, 128:256], b4u, b4l,
                                     start=True, stop=True)        # Nl^8
                    sr8 = recp.tile([128, 256], bf16, name=f"sr8{hc}",
                                    tag="sr8")
                    nc.scalar.copy(sr8[:], pr8[:])
                    R8, b8l = sr8[:, 0:128], sr8[:, 128:256]

                    prm = psw.tile([128, 128], fp32, name=f"prm{hc}", tag="w")
                    nc.tensor.matmul(prm[:], ident, R8, start=True, stop=False)
                    nc.tensor.matmul(prm[:], b8l, R8, start=False, stop=True)
                    sRm = recp.tile([128, 128], bf16, name=f"sRm{hc}",
                                    tag="sRm")
                    nc.vector.tensor_copy(sRm[:], prm[:])

                    # full R^T = Rm (I - G^T);  -G^T = F^T-neg @ Rm
                    pn1 = psw.tile([128, 128], fp32, name=f"pn1{hc}", tag="w")
                    nc.tensor.matmul(pn1[:], mFl[:], sRm[:], start=True,
                                     stop=True)
                    prl = psw.tile([128, 128], bf16, name=f"prl{hc}", tag="w")
                    nc.tensor.transpose(prl[:], sRm[:], ident)
                    sn1 = recp.tile([128, 256], bf16, name=f"sn1{hc}",
                                    tag="sn1")
                    nc.vector.tensor_copy(sn1[:, 0:128], pn1[:])
                    nc.vector.tensor_copy(sn1[:, 128:256], prl[:])
                    n1, Rl = sn1[:, 0:128], sn1[:, 128:256]

                    pRT = psw.tile([128, 128], fp32, name=f"pRT{hc}", tag="w")
                    nc.tensor.matmul(pRT[:], Rl, ident, start=True, stop=False)
                    nc.tensor.matmul(pRT[:], Rl, n1, start=False, stop=True)
                    sRT = recp.tile([128, 128], bf16, name=f"sRT{hc}",
                                    tag="sRT")
                    nc.vector.tensor_copy(sRT[:], pRT[:])

                    # -M^T = -(K^T R^T)  [Dh, C] as two 128-col slices
                    pmt = psw.tile([128, 256], fp32, name=f"pmt{hc}", tag="w")
                    for i in range(2):
                        nc.tensor.matmul(pmt[:, i * 128:(i + 1) * 128],
                                         kcd[:, i * 128:(i + 1) * 128], sRT[:],
                                         start=True, stop=True)
                    smtn = recp.tile([128, 256], bf16, name=f"smtn{hc}",
                                     tag="smtn")
                    nc.vector.tensor_scalar_mul(smtn[:], pmt[:], -1.0)

                    # U' = R V - M S_prev   (psum accumulation; MS on-path)
                    pu = psw.tile([128, 256], fp32, name=f"pu{hc}", tag="w")
                    nc.tensor.matmul(pu[:], sRT[:], vcd, start=True,
                                     stop=False)
                    for i in range(2):
                        nc.tensor.matmul(pu[:], smtn[:, i * 128:(i + 1) * 128],
                                         s_sb[head][:, i * 256:(i + 1) * 256],
                                         start=False, stop=(i == 1))
                    u_sb = recp.tile([128, 256], bf16, name=f"u{hc}", tag="u")
                    nc.vector.tensor_copy(u_sb[:], pu[:])

                    # attn P = triu_incl(K Q^T)
                    pkq = psw.tile([128, 128], fp32, name=f"pkq{hc}", tag="w")
                    for i in range(2):
                        nc.tensor.matmul(pkq[:], KTt[i], QT[i], start=(i == 0),
                                         stop=(i == 1))
                    sPat = recp.tile([128, 128], bf16, name=f"Pat{hc}",
                                     tag="Pat")
                    nc.vector.tensor_mul(sPat[:], pkq[:], m_triuI)

                    # O = Q S_prev + P^T U'
                    po = psw.tile([128, 256], fp32, name=f"po{hc}", tag="w")
                    for i in range(2):
                        nc.tensor.matmul(po[:], QT[i],
                                         s_sb[head][:, i * 256:(i + 1) * 256],
                                         start=(i == 0), stop=False)
                    nc.tensor.matmul(po[:], sPat[:], u_sb[:], start=False,
                                     stop=True)

                    # S += K^T U'   (persistent psum accumulator)
                    for i in range(2):
                        # start only on the very first write: PE "start"
                        # flags the whole 2KB bank pending-zero, so a second
                        # start would make the next chunk's accumulate into
                        # the other half overwrite instead of add.
                        nc.tensor.matmul(s_ps[head][:, i * 256:(i + 1) * 256],
                                         kcd[:, i * 128:(i + 1) * 128], u_sb[:],
                                         start=(ch == 0 and i == 0), stop=True,
                                         skip_group_check=True)
                    s_nb = recs.tile([128, 512], bf16, name=f"ssb{hc}",
                                     tag=f"ssb{head}", bufs=2)
                    nc.vector.tensor_copy(s_nb[:, 0:256], s_ps[head][:, 0:256])
                    nc.scalar.copy(s_nb[:, 256:512],
                                   s_ps[head][:, 256:512])
                    s_sb[head] = s_nb

                    # RMSNorm rows of O, transpose into oT[ch]
                    osq = recp.tile([128, 256], bf16, name=f"osq{hc}",
                                    tag="osq")
                    ossq = recp.tile([128, 1], fp32, name=f"ossq{hc}",
                                     tag="ossq")
                    nc.scalar.activation(osq[:], po[:], AF.Square,
                                         accum_out=ossq[:])
                    orsq = recp.tile([128, 1], fp32, name=f"orsq{hc}",
                                     tag="orsq")
                    nc.scalar.activation(orsq[:], ossq[:], AF.Sqrt,
                                         bias=biases[:, 1:2], scale=1.0 / DH)
                    nc.vector.reciprocal(orsq[:], orsq[:])
                    onrm = recp.tile([128, 256], bf16, name=f"onrm{hc}",
                                     tag="onrm")
                    nc.vector.tensor_scalar_mul(onrm[:], po[:], orsq[:])
                    pto = psw.tile([128, 256], bf16, name=f"pto{hc}", tag="w")
                    for i in range(2):
                        nc.tensor.transpose(pto[:, i * 128:(i + 1) * 128],
                                            onrm[:, i * 128:(i + 1) * 128],
                                            ident)
                    nc.scalar.copy(oTt[:, ct0 * 128:ct0 * 128 + 256], pto[:])

                # ---- phase C for token tile ch (overlaps next chunks) ----
                for half in range(2):
                    pf = psw.tile([128, 512], fp32, name=f"pf{ch}{half}",
                                  tag="w")
                    for ct in range(CT):
                        nc.tensor.matmul(
                            pf[:], oTt[:, ct * 128:(ct + 1) * 128],
                            wo_s[ct][:, half * 512:(half + 1) * 512],
                            start=(ct == 0), stop=(ct == CT - 1))
                    of = ofp.tile([128, 512], fp32, name=f"of{ch}{half}",
                                  tag="of")
                    nc.scalar.copy(of[:], pf[:])
                    nc.sync.dma_start(
                        out_t[ch][:, half * 512:(half + 1) * 512], of[:])


LP_NP = np.float16  # host-side 16-bit dtype matching the device dtype


def _make_consts():
    ii = np.arange(128)
    blk = ii[:, None] // 64 == ii[None, :] // 64
    low = ii[:, None] > ii[None, :]
    nln = -((low & blk).astype(np.float32))
    nun = nln.T.copy()
    fln = -((low & ~blk).astype(np.float32))
    triuI = (ii[:, None] <= ii[None, :]).astype(np.float32)
    ident = np.eye(128, dtype=np.float32)
    return np.concatenate([ident, nln, nun, fln, triuI],
                          axis=1).astype(LP_NP)


def _get_compiled():
    key = ("nc", SILU_NATIVE)
    if key not in _CACHE:
        _CACHE[key] = _build_bass()
    return _CACHE[key]


def kernel(hidden_states, Wq, Wk, Wv, conv_wq, conv_wk, conv_wv, onorm_w, Wo):
    from concourse.bass_utils import run_bass_kernel_spmd

    hidden_states = np.asarray(hidden_states, np.float32)
    Wq = np.asarray(Wq, np.float32)
    Wk = np.asarray(Wk, np.float32)
    Wv = np.asarray(Wv, np.float32)
    Wo = np.asarray(Wo, np.float32)
    conv_wq = np.asarray(conv_wq, np.float32)
    conv_wk = np.asarray(conv_wk, np.float32)
    conv_wv = np.asarray(conv_wv, np.float32)
    onorm_w = np.asarray(onorm_w, np.float32)

    bf = LP_NP
    consts = _make_consts()
    Wo_eff = (Wo * np.tile(onorm_w, H)[:, None]).astype(bf)  # fold RMS weight

    in_maps = []
    for core in range(NCORES):
        b, g = divmod(core, 2)
        cols = slice(CG * g, CG * (g + 1))
        in_maps.append({
            "xT": np.ascontiguousarray(hidden_states[b].T).astype(bf),
            "wq": np.ascontiguousarray(Wq[:, cols]).astype(bf),
            "wk": np.ascontiguousarray(Wk[:, cols]).astype(bf),
            "wv": np.ascontiguousarray(Wv[:, cols]).astype(bf),
            "wo": np.ascontiguousarray(Wo_eff[cols, :]),
            "cw": np.ascontiguousarray(np.concatenate(
                [conv_wq[cols], conv_wk[cols], conv_wv[cols]], axis=1)),
            "consts": consts,
        })

    nc = _get_compiled()
    res = run_bass_kernel_spmd(nc, in_maps, core_ids=list(range(NCORES)),
                               **_CACHE.get("run_kwargs", {}))
    _CACHE["last_results"] = res
    out = np.zeros((B, T, D), np.float32)
    for core in range(NCORES):
        out[core // 2] += res.results[core]["out"]
    return out


# revision 8
# speedup vs baseline: 1.3825x; 1.1327x over previous
"""DeltaNet forward kernel for 8 Trainium2 NeuronCores.

Problem (hardcoded from the task spec): hidden_states [B=4, T=2048, D=1024],
H=4 heads, Dh=256, causal depthwise conv K=4 + silu on q/k/v projections,
q/k l2-normalized per head (q scaled Dh^-0.5), delta-rule recurrence over T,
per-head RMSNorm, merge heads, out = o @ Wo.

Sharding: data-parallel over (batch, head-group): core c -> batch c//2,
head group c%2 (projection columns [512*(c%2), 512*(c%2)+512)). Each core
computes a partial product against its 512 rows of Wo; the host sums the two
partials per batch (the unshard step for the row-parallel output matmul).

Device algorithm (decoupled WY form, chunk C=128): per chunk and head all
S-independent matrices are precomputed off the critical path:
  KK = K K^T;  R_bd^T = (I+B)^{-1} (B = strict upper of KK in 64-blocks)
  via a power ladder exact to B^15;  R^T = R_bd^T (I - G^T) with
  G = R_bd F (2 blocks of 64 -> G^2 = 0, exact);  M^T = K^T R^T.
The sequential S-chain is only:  U' = R V - M S_prev  (psum-accumulated),
S += K^T U', plus two psum->sbuf copies. O = Q S_prev + triu(K Q^T)^T U',
then per-head RMSNorm and transpose into the output-projection layout.
The Wo projection for token tile tt is emitted right after chunk tt so it
overlaps the recurrence. S accumulates in PSUM f32; matmul operands fp16.
"""

import numpy as np

B, T, D = 4, 2048, 1024
H = 4
DH = D // H          # 256
CONV_K = 4
EPS = 1e-5
NCORES = 8
CG = 512             # columns per core (2 heads)
C = 128              # recurrence chunk length
NCHUNK = T // C      # 16
PAD = 4              # front zero padding on time axis for the causal conv
TOKB = 512           # token block (matmul moving size)
KT = D // 128        # 8 contraction tiles
CT = CG // 128       # 4 column tiles per core
NB = T // TOKB       # 4 token blocks

_CACHE = {}
SILU_NATIVE = True  # CoreSim lacks Silu; set False for simulation runs


def _build_bass():
    import concourse.bass as bass  # noqa: F401
    import concourse.bacc as bacc
    import concourse.mybir as mybir
    import concourse.tile as tile

    dt = mybir.dt
    nc = bacc.Bacc("TRN2", target_bir_lowering=False, debug=False)

    xT = nc.dram_tensor("xT", [D, T], dt.float16, kind="ExternalInput")
    wq = nc.dram_tensor("wq", [D, CG], dt.float16, kind="ExternalInput")
    wk = nc.dram_tensor("wk", [D, CG], dt.float16, kind="ExternalInput")
    wv = nc.dram_tensor("wv", [D, CG], dt.float16, kind="ExternalInput")
    wo = nc.dram_tensor("wo", [CG, D], dt.float16, kind="ExternalInput")
    cw = nc.dram_tensor("cw", [CG, 3 * CONV_K], dt.float32, kind="ExternalInput")
    consts = nc.dram_tensor("consts", [128, 5 * 128], dt.float16,
                            kind="ExternalInput")
    out = nc.dram_tensor("out", [T, D], dt.float32, kind="ExternalOutput")

    with tile.TileContext(nc) as tc:
        _body(nc, tc, mybir, xT, wq, wk, wv, wo, cw, consts, out)

    nc.compile()
    return nc


def _body(nc, tc, mybir, xT, wq, wk, wv, wo, cw, consts, out):
    dt = mybir.dt
    AF = mybir.ActivationFunctionType
    ALU = mybir.AluOpType
    fp32 = dt.float32
    bf16 = dt.float16  # 16-bit working dtype (fp16: 11-bit mantissa)
    NT = T + PAD

    xT_t = xT.ap().rearrange("(n p) t -> n p t", p=128)       # [8,128,T]
    w_t = {"q": wq.ap().rearrange("(n p) c -> n p c", p=128),
           "k": wk.ap().rearrange("(n p) c -> n p c", p=128),
           "v": wv.ap().rearrange("(n p) c -> n p c", p=128)}
    wo_t = wo.ap().rearrange("(n p) c -> n p c", p=128)       # [4,128,D]
    cw_t = cw.ap().rearrange("(n p) c -> n p c", p=128)       # [4,128,12]
    out_t = out.ap().rearrange("(n p) c -> n p c", p=128)     # [16,128,D]

    # ---------- persistent pool (lives for the whole kernel) ----------
    with tc.tile_pool(name="persist", bufs=1) as persist, \
         tc.tile_pool(name="qkvp", bufs=3 * CT) as qkvp, \
         tc.tile_pool(name="otp", bufs=6) as otp, \
         tc.tile_pool(name="wop", bufs=CT) as wop, \
         tc.tile_pool(name="psw", bufs=2, space="PSUM") as psw, \
         tc.tile_pool(name="pss", bufs=1, space="PSUM") as pss:

        cons = persist.tile([128, 5 * 128], bf16, name="cons", tag="cons")
        nc.sync.dma_start(cons[:], consts.ap())
        ident = cons[:, 0:128]          # identity
        m_nln = cons[:, 128:256]        # -(strict lower, in 64-block)
        m_nun = cons[:, 256:384]        # -(strict upper, in 64-block)
        m_fln = cons[:, 384:512]        # -(strict lower, outside 64-blocks)
        m_triuI = cons[:, 512:640]      # i<=j, +1
        ones_col = cons[:, 639:640]     # last col of (i<=j) mask == all ones

        biases = persist.tile([128, 3], dt.float32, name="biases", tag="biases")
        nc.vector.memset(biases[:, 0:1], 1e-6)
        nc.vector.memset(biases[:, 1:2], EPS)
        nc.vector.memset(biases[:, 2:3], 1e-6 * DH)

        wo_s = []
        for ct in range(CT):
            t_ = wop.tile([128, D], bf16, name=f"wo{ct}", tag="wo")
            nc.sync.dma_start(t_[:], wo_t[ct])
            wo_s.append(t_)

        cwt = []
        for ct in range(CT):
            t_ = persist.tile([128, 3 * CONV_K], fp32, name=f"cw{ct}",
                              tag=f"cw{ct}")
            nc.sync.dma_start(t_[:], cw_t[ct])
            cwt.append(t_)

        qh, kh, vh = [], [], []
        for lst, nm in ((qh, "q"), (kh, "k"), (vh, "v")):
            for ct in range(CT):
                lst.append(qkvp.tile([128, T], bf16, name=f"{nm}hat{ct}",
                                     tag="qkv"))

        # ================= phase A: projections + conv + silu + l2norm ====
        with tc.tile_pool(name="xp", bufs=KT) as xp, \
             tc.tile_pool(name="wp", bufs=3 * KT) as wp, \
             tc.tile_pool(name="rawp", bufs=2) as rawp, \
             tc.tile_pool(name="sqp", bufs=4) as sqp, \
             tc.tile_pool(name="stp", bufs=1) as stp, \
             tc.tile_pool(name="bcp", bufs=2) as bcp:

            xt = []
            for kt in range(KT):
                t_ = xp.tile([128, T], bf16, name=f"xt{kt}", tag="xt")
                nc.sync.dma_start(t_[:], xT_t[kt])
                xt.append(t_)
            ws = {}
            for nm in ("q", "k", "v"):
                ws[nm] = []
                for kt in range(KT):
                    t_ = wp.tile([128, CG], bf16, name=f"w{nm}{kt}", tag="w")
                    nc.sync.dma_start(t_[:], w_t[nm][kt])
                    ws[nm].append(t_)

            for ti, (nm, dest) in enumerate((("q", qh), ("k", kh), ("v", vh))):
                sq_tiles = []
                for ct in range(CT):
                    rawt = rawp.tile([128, NT], bf16, name=f"raw{nm}{ct}",
                                     tag="raw")
                    nc.vector.memset(rawt[:, 0:PAD], 0.0)
                    dst = dest[ct]
                    for nb in range(NB):
                        pt = psw.tile([128, TOKB], fp32, name=f"pp{nm}{ct}{nb}",
                                      tag="lad" if nb % 2 else "tr")
                        for kt in range(KT):
                            nc.tensor.matmul(
                                pt[:], ws[nm][kt][:, ct * 128:(ct + 1) * 128],
                                xt[kt][:, nb * TOKB:(nb + 1) * TOKB],
                                start=(kt == 0), stop=(kt == KT - 1))
                        nc.scalar.copy(
                            rawt[:, PAD + nb * TOKB:PAD + (nb + 1) * TOKB],
                            pt[:])
                    # causal depthwise conv along t
                    w0 = cwt[ct][:, ti * CONV_K:ti * CONV_K + 1]
                    nc.vector.tensor_scalar_mul(dst[:], rawt[:, 1:1 + T], w0)
                    for i in range(1, CONV_K):
                        wi = cwt[ct][:, ti * CONV_K + i:ti * CONV_K + i + 1]
                        nc.vector.scalar_tensor_tensor(
                            dst[:], rawt[:, 1 + i:1 + i + T], wi, dst[:],
                            ALU.mult, ALU.add)
                    if SILU_NATIVE:
                        nc.scalar.activation(dst[:], dst[:], AF.Silu)
                    else:
                        sg = rawp.tile([128, T], bf16, name=f"sg{nm}{ct}",
                                       tag="raw")
                        nc.scalar.activation(sg[:], dst[:], AF.Sigmoid)
                        nc.vector.tensor_mul(dst[:], dst[:], sg[:])
                    if ti < 2:
                        sqt = sqp.tile([128, T], bf16, name=f"sq{nm}{ct}",
                                       tag="sq")
                        nc.scalar.activation(sqt[:], dst[:], AF.Square)
                        sq_tiles.append(sqt)
                if ti < 2:
                    # per-head l2norm: sumsq rows via ones-matmul, broadcast
                    # to 128 partitions, rsq = scale/sqrt(ss + 1e-6), apply.
                    for head in range(2):
                        bcf = bcp.tile([128, T], fp32, name=f"bcf{nm}{head}",
                                       tag="bcf")
                        for nb in range(NB):
                            prow = psw.tile([1, TOKB], fp32,
                                            name=f"pr{nm}{head}{nb}",
                                            tag="upo")
                            for cth in range(2):
                                nc.tensor.matmul(
                                    prow[:], ones_col,
                                    sq_tiles[head * 2 + cth][
                                        :, nb * TOKB:(nb + 1) * TOKB],
                                    start=(cth == 0), stop=(cth == 1))
                            rowb = stp.tile([1, TOKB], fp32,
                                            name=f"rb{nm}{head}{nb}",
                                            tag="rowb", bufs=3)
                            nc.scalar.copy(rowb[:], prow[:])
                            nc.gpsimd.partition_broadcast(
                                bcf[:, nb * TOKB:(nb + 1) * TOKB], rowb[:])
                        if ti == 0:
                            # fold Dh^-0.5: 1/(16 sqrt(ss+eps)) =
                            # 1/sqrt(256 ss + 256 eps)
                            nc.scalar.activation(bcf[:], bcf[:], AF.Sqrt,
                                                 bias=biases[:, 2:3],
                                                 scale=float(DH))
                        else:
                            nc.scalar.activation(bcf[:], bcf[:], AF.Sqrt,
                                                 bias=biases[:, 0:1])
                        nc.vector.reciprocal(bcf[:], bcf[:])
                        bcb = bcp.tile([128, T], bf16, name=f"bcb{nm}{head}",
                                       tag="bcb")
                        nc.scalar.copy(bcb[:], bcf[:])
                        for cth in range(2):
                            ct = head * 2 + cth
                            nc.vector.tensor_mul(dest[ct][:], dest[ct][:],
                                                 bcb[:])

        # ================= phase B: delta-rule recurrence + phase C =======
        with tc.tile_pool(name="recp", bufs=4) as recp, \
             tc.tile_pool(name="recs", bufs=1) as recs, \
             tc.tile_pool(name="ofp", bufs=3) as ofp:
            s_ps, s_sb = {}, {}
            for head in range(2):
                s_ps[head] = pss.tile([128, 512], fp32, name=f"sps{head}",
                                      tag=f"sps{head}")
                s_sb[head] = recs.tile([128, 512], bf16, name=f"ssb{head}i",
                                       tag=f"ssb{head}", bufs=2)
                nc.vector.memset(s_sb[head][:], 0.0)

            for ch in range(NCHUNK):
                t0 = ch * C
                oTt = otp.tile([128, 512], bf16, name=f"oT{ch}", tag="oT")
                for head in range(2):
                    ct0 = head * 2
                    QT = [qh[ct0][:, t0:t0 + C], qh[ct0 + 1][:, t0:t0 + C]]
                    KTt = [kh[ct0][:, t0:t0 + C], kh[ct0 + 1][:, t0:t0 + C]]
                    VT = [vh[ct0][:, t0:t0 + C], vh[ct0 + 1][:, t0:t0 + C]]
                    hc = f"{head}_{ch}"

                    # K, V in [C, Dh] via PE transpose into one psum bank
                    pkv = psw.tile([128, 512], bf16, name=f"pkv{hc}", tag="tr")
                    for i in range(2):
                        nc.tensor.transpose(pkv[:, i * 128:(i + 1) * 128],
                                            KTt[i], ident)
                        nc.tensor.transpose(pkv[:, 256 + i * 128:384 + i * 128],
                                            VT[i], ident)
                    kvcd = recp.tile([128, 512], bf16, name=f"kvcd{hc}",
                                     tag="kvcd")
                    nc.scalar.copy(kvcd[:], pkv[:])
                    kcd = kvcd[:, 0:256]
                    vcd = kvcd[:, 256:512]

                    # KK^T and masked pieces; lada packs [pkk | b2,b2l | T1]
                    lada = psw.tile([128, 512], fp32, name=f"lada{hc}",
                                    tag="lad")
                    pkk = lada[:, 0:128]
                    for i in range(2):
                        nc.tensor.matmul(pkk[:], KTt[i], KTt[i], start=(i == 0),
                                         stop=(i == 1))
                    mNl = recp.tile([128, 128], bf16, name=f"mNl{hc}",
                                    tag="mNl")
                    mNu = recp.tile([128, 128], bf16, name=f"mNu{hc}",
                                    tag="mNu")
                    mFl = recp.tile([128, 128], bf16, name=f"mFl{hc}",
                                    tag="mFl")
                    nc.vector.tensor_mul(mNl[:], pkk[:], m_nln)    # -B^T
                    nc.vector.tensor_mul(mNu[:], pkk[:], m_nun)    # -B
                    nc.vector.tensor_mul(mFl[:], pkk[:], m_fln)    # -F

                    # ladder: Rm = sum_{j<=15} (-B)^j  (B strict upper in-64)
                    pl = lada[:, 128:384]
                    nc.tensor.matmul(pl[:, 0:128], mNl[:], mNu[:],
                                     start=True, stop=True)        # B^2
                    nc.tensor.matmul(pl[:, 128:256], mNu[:], mNl[:],
                                     start=True, stop=True)        # Nl^2
                    sb2 = recp.tile([128, 256], bf16, name=f"sb2{hc}",
                                    tag="sb2")
                    nc.scalar.copy(sb2[:], pl[:])
                    b2, b2l = sb2[:, 0:128], sb2[:, 128:256]

                    pt1 = lada[:, 384:512]
                    nc.tensor.matmul(pt1[:], ident, ident, start=True,
                                     stop=False)
                    nc.tensor.matmul(pt1[:], ident, mNu[:], start=False,
                                     stop=False)
                    nc.tensor.matmul(pt1[:], ident, b2, start=False,
                                     stop=False)
                    nc.tensor.matmul(pt1[:], mNl[:], b2, start=False,
                                     stop=True)                    # -B^3
                    sT1 = recp.tile([128, 128], bf16, name=f"sT1{hc}",
                                    tag="sT1")
                    nc.scalar.copy(sT1[:], pt1[:])

                    ladb = psw.tile([128, 512], fp32, name=f"ladb{hc}",
                                    tag="lad")
                    pl2 = ladb[:, 0:256]
                    nc.tensor.matmul(pl2[:, 0:128], b2, b2l,
                                     start=True, stop=True)        # Nl^4
                    nc.tensor.matmul(pl2[:, 128:256], b2l, b2,
                                     start=True, stop=True)        # Nu^4
                    sb4 = recp.tile([128, 256], bf16, name=f"sb4{hc}",
                                    tag="sb4")
                    nc.scalar.copy(sb4[:], pl2[:])
                    b4l, b4u = sb4[:, 0:128], sb4[:, 128:256]

                    pr8 = ladb[:, 256:512]
                    nc.tensor.matmul(pr8[:, 0:128], ident, sT1[:],
                                     start=True, stop=False)
                    nc.tensor.matmul(pr8[:, 0:128], b4l, sT1[:],
                                     start=False, stop=True)       # (I+B^4)T1
                    nc.tensor.matmul(pr8[:, 128:256], b4u, b4l,
                                     start=True, stop=True)        # Nl^8
                    sr8 = recp.tile([128, 256], bf16, name=f"sr8{hc}",
                                    tag="sr8")
                    nc.scalar.copy(sr8[:], pr8[:])
                    R8, b8l = sr8[:, 0:128], sr8[:, 128:256]

                    ladc = psw.tile([128, 512], fp32, name=f"ladc{hc}",
                                    tag="lad")
                    prm = ladc[:, 0:128]
                    nc.tensor.matmul(prm[:], ident, R8, start=True, stop=False)
                    nc.tensor.matmul(prm[:], b8l, R8, start=False, stop=True)
                    sRm = recp.tile([128, 128], bf16, name=f"sRm{hc}",
                                    tag="sRm")
                    nc.vector.tensor_copy(sRm[:], prm[:])

                    # full R^T = Rm (I - G^T);  -G^T = F^T-neg @ Rm
                    pn1 = ladc[:, 128:256]
                    nc.tensor.matmul(pn1[:], mFl[:], sRm[:], start=True,
                                     stop=True)
                    ptr2 = psw.tile([128, 384], bf16, name=f"ptr2{hc}",
                                    tag="tr")
                    prl = ptr2[:, 0:128]
                    nc.tensor.transpose(prl[:], sRm[:], ident)
                    sn1 = recp.tile([128, 256], bf16, name=f"sn1{hc}",
                                    tag="sn1")
                    nc.vector.tensor_copy(sn1[:, 0:128], pn1[:])
                    nc.vector.tensor_copy(sn1[:, 128:256], prl[:])
                    n1, Rl = sn1[:, 0:128], sn1[:, 128:256]

                    pRT = ladc[:, 256:384]
                    nc.tensor.matmul(pRT[:], Rl, ident, start=True, stop=False)
                    nc.tensor.matmul(pRT[:], Rl, n1, start=False, stop=True)
                    sRT = recp.tile([128, 128], bf16, name=f"sRT{hc}",
                                    tag="sRT")
                    nc.vector.tensor_copy(sRT[:], pRT[:])

                    # -M^T = -(K^T R^T)  [Dh, C] as two 128-col slices
                    pmt = psw.tile([128, 256], fp32, name=f"pmt{hc}",
                                   tag="lad")
                    for i in range(2):
                        nc.tensor.matmul(pmt[:, i * 128:(i + 1) * 128],
                                         kcd[:, i * 128:(i + 1) * 128], sRT[:],
                                         start=True, stop=True)
                    smtn = recp.tile([128, 256], bf16, name=f"smtn{hc}",
                                     tag="smtn")
                    nc.vector.tensor_scalar_mul(smtn[:], pmt[:], -1.0)

                    # U' = R V - M S_prev; upo packs [pu | po]
                    upo = psw.tile([128, 512], fp32, name=f"upo{hc}",
                                   tag="upo")
                    pu = upo[:, 0:256]
                    nc.tensor.matmul(pu[:], sRT[:], vcd, start=True,
                                     stop=False)
                    for i in range(2):
                        nc.tensor.matmul(pu[:], smtn[:, i * 128:(i + 1) * 128],
                                         s_sb[head][:, i * 256:(i + 1) * 256],
                                         start=False, stop=(i == 1))
                    u_sb = recp.tile([128, 256], bf16, name=f"u{hc}", tag="u")
                    nc.vector.tensor_copy(u_sb[:], pu[:])

                    # attn P = triu_incl(K Q^T)
                    pkq = ladc[:, 384:512]
                    for i in range(2):
                        nc.tensor.matmul(pkq[:], KTt[i], QT[i], start=(i == 0),
                                         stop=(i == 1))
                    sPat = recp.tile([128, 128], bf16, name=f"Pat{hc}",
                                     tag="Pat")
                    nc.vector.tensor_mul(sPat[:], pkq[:], m_triuI)

                    # O = Q S_prev + P^T U'
                    po = upo[:, 256:512]
                    for i in range(2):
                        nc.tensor.matmul(po[:], QT[i],
                                         s_sb[head][:, i * 256:(i + 1) * 256],
                                         start=(i == 0), stop=False)
                    nc.tensor.matmul(po[:], sPat[:], u_sb[:], start=False,
                                     stop=True)

                    # S += K^T U'   (persistent psum accumulator)
                    for i in range(2):
                        # start only on the very first write: PE "start"
                        # flags the whole 2KB bank pending-zero, so a second
                        # start would make the next chunk's accumulate into
                        # the other half overwrite instead of add.
                        nc.tensor.matmul(s_ps[head][:, i * 256:(i + 1) * 256],
                                         kcd[:, i * 128:(i + 1) * 128], u_sb[:],
                                         start=(ch == 0 and i == 0), stop=True,
                                         skip_group_check=True)
                    s_nb = recs.tile([128, 512], bf16, name=f"ssb{hc}",
                                     tag=f"ssb{head}", bufs=2)
                    nc.vector.tensor_copy(s_nb[:, 0:256], s_ps[head][:, 0:256])
                    nc.scalar.copy(s_nb[:, 256:512],
                                   s_ps[head][:, 256:512])
                    s_sb[head] = s_nb

                    # RMSNorm rows of O, transpose into oT[ch]
                    osq = recp.tile([128, 256], bf16, name=f"osq{hc}",
                                    tag="osq")
                    ossq = recp.tile([128, 1], fp32, name=f"ossq{hc}",
                                     tag="ossq")
                    nc.scalar.activation(osq[:], po[:], AF.Square,
                                         accum_out=ossq[:])
                    orsq = recp.tile([128, 1], fp32, name=f"orsq{hc}",
                                     tag="orsq")
                    nc.scalar.activation(orsq[:], ossq[:], AF.Sqrt,
                                         bias=biases[:, 1:2], scale=1.0 / DH)
                    nc.vector.reciprocal(orsq[:], orsq[:])
                    onrm = recp.tile([128, 256], bf16, name=f"onrm{hc}",
                                     tag="onrm")
                    nc.vector.tensor_scalar_mul(onrm[:], po[:], orsq[:])
                    pto = ptr2[:, 128:384]
                    for i in range(2):
                        nc.tensor.transpose(pto[:, i * 128:(i + 1) * 128],
                                            onrm[:, i * 128:(i + 1) * 128],
                                            ident)
                    nc.scalar.copy(oTt[:, ct0 * 128:ct0 * 128 + 256], pto[:])

                # ---- phase C for token tile ch (overlaps next chunks) ----
                for half in range(2):
                    pf = psw.tile([128, 512], fp32, name=f"pf{ch}{half}",
                                  tag="upo")
                    for ct in range(CT):
                        nc.tensor.matmul(
                            pf[:], oTt[:, ct * 128:(ct + 1) * 128],
                            wo_s[ct][:, half * 512:(half + 1) * 512],
                            start=(ct == 0), stop=(ct == CT - 1))
                    of = ofp.tile([128, 512], fp32, name=f"of{ch}{half}",
                                  tag="of")
                    nc.scalar.copy(of[:], pf[:])
                    nc.sync.dma_start(
                        out_t[ch][:, half * 512:(half + 1) * 512], of[:])


LP_NP = np.float16  # host-side 16-bit dtype matching the device dtype


def _make_consts():
    ii = np.arange(128)
    blk = ii[:, None] // 64 == ii[None, :] // 64
    low = ii[:, None] > ii[None, :]
    nln = -((low & blk).astype(np.float32))
    nun = nln.T.copy()
    fln = -((low & ~blk).astype(np.float32))
    triuI = (ii[:, None] <= ii[None, :]).astype(np.float32)
    ident = np.eye(128, dtype=np.float32)
    return np.concatenate([ident, nln, nun, fln, triuI],
                          axis=1).astype(LP_NP)


def _get_compiled():
    key = ("nc", SILU_NATIVE)
    if key not in _CACHE:
        _CACHE[key] = _build_bass()
    return _CACHE[key]


def kernel(hidden_states, Wq, Wk, Wv, conv_wq, conv_wk, conv_wv, onorm_w, Wo):
    from concourse.bass_utils import run_bass_kernel_spmd

    hidden_states = np.asarray(hidden_states, np.float32)
    Wq = np.asarray(Wq, np.float32)
    Wk = np.asarray(Wk, np.float32)
    Wv = np.asarray(Wv, np.float32)
    Wo = np.asarray(Wo, np.float32)
    conv_wq = np.asarray(conv_wq, np.float32)
    conv_wk = np.asarray(conv_wk, np.float32)
    conv_wv = np.asarray(conv_wv, np.float32)
    onorm_w = np.asarray(onorm_w, np.float32)

    bf = LP_NP
    consts = _make_consts()
    Wo_eff = (Wo * np.tile(onorm_w, H)[:, None]).astype(bf)  # fold RMS weight

    in_maps = []
    for core in range(NCORES):
        b, g = divmod(core, 2)
        cols = slice(CG * g, CG * (g + 1))
        in_maps.append({
            "xT": np.ascontiguousarray(hidden_states[b].T).astype(bf),
            "wq": np.ascontiguousarray(Wq[:, cols]).astype(bf),
            "wk": np.ascontiguousarray(Wk[:, cols]).astype(bf),
            "wv": np.ascontiguousarray(Wv[:, cols]).astype(bf),
            "wo": np.ascontiguousarray(Wo_eff[cols, :]),
            "cw": np.ascontiguousarray(np.concatenate(
                [conv_wq[cols], conv_wk[cols], conv_wv[cols]], axis=1)),
            "consts": consts,
        })

    nc = _get_compiled()
    res = run_bass_kernel_spmd(nc, in_maps, core_ids=list(range(NCORES)),
                               **_CACHE.get("run_kwargs", {}))
    _CACHE["last_results"] = res
    out = np.zeros((B, T, D), np.float32)
    for core in range(NCORES):
        out[core // 2] += res.results[core]["out"]
    return out


# revision 9
# speedup vs baseline: 1.7621x; 1.2745x over previous
"""DeltaNet forward kernel for 8 Trainium2 NeuronCores.

Problem (hardcoded from the task spec): hidden_states [B=4, T=2048, D=1024],
H=4 heads, Dh=256, causal depthwise conv K=4 + silu on q/k/v projections,
q/k l2-normalized per head (q scaled Dh^-0.5), delta-rule recurrence over T,
per-head RMSNorm, merge heads, out = o @ Wo.

Sharding: data-parallel over (batch, head-group): core c -> batch c//2,
head group c%2 (projection columns [512*(c%2), 512*(c%2)+512)). Each core
computes a partial product against its 512 rows of Wo; the host sums the two
partials per batch (the unshard step for the row-parallel output matmul).

Device algorithm (decoupled WY form, chunk C=128): per chunk and head all
S-independent matrices are precomputed off the critical path:
  KK = K K^T;  R_bd^T = (I+B)^{-1} (B = strict upper of KK in 64-blocks)
  via a power ladder exact to B^15;  R^T = R_bd^T (I - G^T) with
  G = R_bd F (2 blocks of 64 -> G^2 = 0, exact);  M^T = K^T R^T.
The sequential S-chain is only:  U' = R V - M S_prev  (psum-accumulated),
S += K^T U', plus two psum->sbuf copies. O = Q S_prev + triu(K Q^T)^T U',
then per-head RMSNorm and transpose into the output-projection layout.
The Wo projection for token tile tt is emitted right after chunk tt so it
overlaps the recurrence. S accumulates in PSUM f32; matmul operands fp16.
"""

import numpy as np

B, T, D = 4, 2048, 1024
H = 4
DH = D // H          # 256
CONV_K = 4
EPS = 1e-5
NCORES = 8
CG = 512             # columns per core (2 heads)
C = 128              # recurrence chunk length
NCHUNK = T // C      # 16
PAD = 4              # front zero padding on time axis for the causal conv
TOKB = 512           # token block (matmul moving size)
KT = D // 128        # 8 contraction tiles
CT = CG // 128       # 4 column tiles per core
NB = T // TOKB       # 4 token blocks

_CACHE = {}
SILU_NATIVE = True  # CoreSim lacks Silu; set False for simulation runs


def _build_bass():
    import concourse.bass as bass  # noqa: F401
    import concourse.bacc as bacc
    import concourse.mybir as mybir
    import concourse.tile as tile

    dt = mybir.dt
    nc = bacc.Bacc("TRN2", target_bir_lowering=False, debug=False)

    xT = nc.dram_tensor("xT", [D, T], dt.float16, kind="ExternalInput")
    wq = nc.dram_tensor("wq", [D, CG], dt.float16, kind="ExternalInput")
    wk = nc.dram_tensor("wk", [D, CG], dt.float16, kind="ExternalInput")
    wv = nc.dram_tensor("wv", [D, CG], dt.float16, kind="ExternalInput")
    wo = nc.dram_tensor("wo", [CG, D], dt.float16, kind="ExternalInput")
    cw = nc.dram_tensor("cw", [CG, 3 * CONV_K], dt.float32, kind="ExternalInput")
    consts = nc.dram_tensor("consts", [128, 5 * 128], dt.float16,
                            kind="ExternalInput")
    out = nc.dram_tensor("out", [T, D], dt.float32, kind="ExternalOutput")

    with tile.TileContext(nc) as tc:
        _body(nc, tc, mybir, xT, wq, wk, wv, wo, cw, consts, out)

    nc.compile()
    return nc


def _body(nc, tc, mybir, xT, wq, wk, wv, wo, cw, consts, out):
    dt = mybir.dt
    AF = mybir.ActivationFunctionType
    ALU = mybir.AluOpType
    fp32 = dt.float32
    bf16 = dt.float16  # 16-bit working dtype (fp16: 11-bit mantissa)
    NT = T + PAD

    xT_t = xT.ap().rearrange("(n p) t -> n p t", p=128)       # [8,128,T]
    w_t = {"q": wq.ap().rearrange("(n p) c -> n p c", p=128),
           "k": wk.ap().rearrange("(n p) c -> n p c", p=128),
           "v": wv.ap().rearrange("(n p) c -> n p c", p=128)}
    wo_t = wo.ap().rearrange("(n p) c -> n p c", p=128)       # [4,128,D]
    cw_t = cw.ap().rearrange("(n p) c -> n p c", p=128)       # [4,128,12]
    out_t = out.ap().rearrange("(n p) c -> n p c", p=128)     # [16,128,D]

    # ---------- persistent pool (lives for the whole kernel) ----------
    with tc.tile_pool(name="persist", bufs=1) as persist, \
         tc.tile_pool(name="qkvp", bufs=3 * CT) as qkvp, \
         tc.tile_pool(name="otp", bufs=6) as otp, \
         tc.tile_pool(name="wop", bufs=CT) as wop, \
         tc.tile_pool(name="psw", bufs=2, space="PSUM") as psw, \
         tc.tile_pool(name="pss", bufs=1, space="PSUM") as pss:

        cons = persist.tile([128, 5 * 128], bf16, name="cons", tag="cons")
        nc.sync.dma_start(cons[:], consts.ap())
        ident = cons[:, 0:128]          # identity
        m_nln = cons[:, 128:256]        # -(strict lower, in 64-block)
        m_nun = cons[:, 256:384]        # -(strict upper, in 64-block)
        m_fln = cons[:, 384:512]        # -(strict lower, outside 64-blocks)
        m_triuI = cons[:, 512:640]      # i<=j, +1
        ones_col = cons[:, 639:640]     # last col of (i<=j) mask == all ones

        biases = persist.tile([128, 3], dt.float32, name="biases", tag="biases")
        nc.vector.memset(biases[:, 0:1], 1e-6)
        nc.vector.memset(biases[:, 1:2], EPS)
        nc.vector.memset(biases[:, 2:3], 1e-6 * DH)

        wo_s = []
        for ct in range(CT):
            t_ = wop.tile([128, D], bf16, name=f"wo{ct}", tag="wo")
            nc.sync.dma_start(t_[:], wo_t[ct])
            wo_s.append(t_)

        cwt = []
        for ct in range(CT):
            t_ = persist.tile([128, 3 * CONV_K], fp32, name=f"cw{ct}",
                              tag=f"cw{ct}")
            nc.sync.dma_start(t_[:], cw_t[ct])
            cwt.append(t_)

        qh, kh, vh = [], [], []
        for lst, nm in ((qh, "q"), (kh, "k"), (vh, "v")):
            for ct in range(CT):
                lst.append(qkvp.tile([128, T], bf16, name=f"{nm}hat{ct}",
                                     tag="qkv"))

        # ================= phase A: projections + conv + silu + l2norm ====
        with tc.tile_pool(name="xp", bufs=KT) as xp, \
             tc.tile_pool(name="wp", bufs=3 * KT) as wp, \
             tc.tile_pool(name="rawp", bufs=2) as rawp, \
             tc.tile_pool(name="sqp", bufs=4) as sqp, \
             tc.tile_pool(name="stp", bufs=1) as stp, \
             tc.tile_pool(name="bcp", bufs=2) as bcp:

            xt = []
            for kt in range(KT):
                t_ = xp.tile([128, T], bf16, name=f"xt{kt}", tag="xt")
                nc.sync.dma_start(t_[:], xT_t[kt])
                xt.append(t_)
            ws = {}
            for nm in ("q", "k", "v"):
                ws[nm] = []
                for kt in range(KT):
                    t_ = wp.tile([128, CG], bf16, name=f"w{nm}{kt}", tag="w")
                    nc.sync.dma_start(t_[:], w_t[nm][kt])
                    ws[nm].append(t_)

            for ti, (nm, dest) in enumerate((("q", qh), ("k", kh), ("v", vh))):
                sq_tiles = []
                for ct in range(CT):
                    rawt = rawp.tile([128, NT], bf16, name=f"raw{nm}{ct}",
                                     tag="raw")
                    nc.vector.memset(rawt[:, 0:PAD], 0.0)
                    dst = dest[ct]
                    for nb in range(NB):
                        pt = psw.tile([128, TOKB], fp32, name=f"pp{nm}{ct}{nb}",
                                      tag="lad" if nb % 2 else "tr")
                        for kt in range(KT):
                            nc.tensor.matmul(
                                pt[:], ws[nm][kt][:, ct * 128:(ct + 1) * 128],
                                xt[kt][:, nb * TOKB:(nb + 1) * TOKB],
                                start=(kt == 0), stop=(kt == KT - 1))
                        nc.scalar.copy(
                            rawt[:, PAD + nb * TOKB:PAD + (nb + 1) * TOKB],
                            pt[:])
                    # causal depthwise conv along t
                    w0 = cwt[ct][:, ti * CONV_K:ti * CONV_K + 1]
                    nc.vector.tensor_scalar_mul(dst[:], rawt[:, 1:1 + T], w0)
                    for i in range(1, CONV_K):
                        wi = cwt[ct][:, ti * CONV_K + i:ti * CONV_K + i + 1]
                        nc.vector.scalar_tensor_tensor(
                            dst[:], rawt[:, 1 + i:1 + i + T], wi, dst[:],
                            ALU.mult, ALU.add)
                    if SILU_NATIVE:
                        nc.scalar.activation(dst[:], dst[:], AF.Silu)
                    else:
                        sg = rawp.tile([128, T], bf16, name=f"sg{nm}{ct}",
                                       tag="raw")
                        nc.scalar.activation(sg[:], dst[:], AF.Sigmoid)
                        nc.vector.tensor_mul(dst[:], dst[:], sg[:])
                    if ti < 2:
                        sqt = sqp.tile([128, T], bf16, name=f"sq{nm}{ct}",
                                       tag="sq")
                        nc.scalar.activation(sqt[:], dst[:], AF.Square)
                        sq_tiles.append(sqt)
                if ti < 2:
                    # per-head l2norm: sumsq rows via ones-matmul, broadcast
                    # to 128 partitions, rsq = scale/sqrt(ss + 1e-6), apply.
                    for head in range(2):
                        bcf = bcp.tile([128, T], fp32, name=f"bcf{nm}{head}",
                                       tag="bcf")
                        for nb in range(NB):
                            prow = psw.tile([1, TOKB], fp32,
                                            name=f"pr{nm}{head}{nb}",
                                            tag="upo")
                            for cth in range(2):
                                nc.tensor.matmul(
                                    prow[:], ones_col,
                                    sq_tiles[head * 2 + cth][
                                        :, nb * TOKB:(nb + 1) * TOKB],
                                    start=(cth == 0), stop=(cth == 1))
                            rowb = stp.tile([1, TOKB], fp32,
                                            name=f"rb{nm}{head}{nb}",
                                            tag="rowb", bufs=3)
                            nc.scalar.copy(rowb[:], prow[:])
                            nc.gpsimd.partition_broadcast(
                                bcf[:, nb * TOKB:(nb + 1) * TOKB], rowb[:])
                        if ti == 0:
                            # fold Dh^-0.5: 1/(16 sqrt(ss+eps)) =
                            # 1/sqrt(256 ss + 256 eps)
                            nc.scalar.activation(bcf[:], bcf[:], AF.Sqrt,
                                                 bias=biases[:, 2:3],
                                                 scale=float(DH))
                        else:
                            nc.scalar.activation(bcf[:], bcf[:], AF.Sqrt,
                                                 bias=biases[:, 0:1])
                        nc.vector.reciprocal(bcf[:], bcf[:])
                        bcb = bcp.tile([128, T], bf16, name=f"bcb{nm}{head}",
                                       tag="bcb")
                        nc.scalar.copy(bcb[:], bcf[:])
                        for cth in range(2):
                            ct = head * 2 + cth
                            nc.vector.tensor_mul(dest[ct][:], dest[ct][:],
                                                 bcb[:])

        # ================= phase B: delta-rule recurrence + phase C =======
        with tc.tile_pool(name="recp", bufs=4) as recp, \
             tc.tile_pool(name="recs", bufs=1) as recs, \
             tc.tile_pool(name="ofp", bufs=3) as ofp:
            s_ps, s_sb = {}, {}
            for head in range(2):
                s_ps[head] = pss.tile([128, 512], fp32, name=f"sps{head}",
                                      tag=f"sps{head}")
                s_sb[head] = recs.tile([128, 512], bf16, name=f"ssb{head}i",
                                       tag=f"ssb{head}", bufs=2)
                nc.vector.memset(s_sb[head][:], 0.0)

            for ch in range(NCHUNK):
                t0 = ch * C
                oTt = otp.tile([128, 512], bf16, name=f"oT{ch}", tag="oT")
                for head in range(2):
                    ct0 = head * 2
                    QT = [qh[ct0][:, t0:t0 + C], qh[ct0 + 1][:, t0:t0 + C]]
                    KTt = [kh[ct0][:, t0:t0 + C], kh[ct0 + 1][:, t0:t0 + C]]
                    VT = [vh[ct0][:, t0:t0 + C], vh[ct0 + 1][:, t0:t0 + C]]
                    hc = f"{head}_{ch}"

                    # K, V in [C, Dh] via PE transpose into one psum bank
                    # tr bank packs [pkv 0:512 | prl 512:640 | pto 640:896]
                    ptr = psw.tile([128, 896], bf16, name=f"ptr{hc}", tag="tr")
                    pkv = ptr[:, 0:512]
                    for i in range(2):
                        nc.tensor.transpose(pkv[:, i * 128:(i + 1) * 128],
                                            KTt[i], ident)
                        nc.tensor.transpose(pkv[:, 256 + i * 128:384 + i * 128],
                                            VT[i], ident)
                    kvcd = recp.tile([128, 512], bf16, name=f"kvcd{hc}",
                                     tag="kvcd")
                    nc.scalar.copy(kvcd[:], pkv[:])
                    kcd = kvcd[:, 0:256]
                    vcd = kvcd[:, 256:512]

                    # KK^T and masked pieces; lada packs [pkk | b2,b2l | T1]
                    lada = psw.tile([128, 512], fp32, name=f"lada{hc}",
                                    tag="lad")
                    pkk = lada[:, 0:128]
                    for i in range(2):
                        nc.tensor.matmul(pkk[:], KTt[i], KTt[i], start=(i == 0),
                                         stop=(i == 1))
                    mNl = recp.tile([128, 128], bf16, name=f"mNl{hc}",
                                    tag="mNl")
                    mNu = recp.tile([128, 128], bf16, name=f"mNu{hc}",
                                    tag="mNu")
                    mFl = recp.tile([128, 128], bf16, name=f"mFl{hc}",
                                    tag="mFl")
                    nc.vector.tensor_mul(mNl[:], pkk[:], m_nln)    # -B^T
                    nc.vector.tensor_mul(mNu[:], pkk[:], m_nun)    # -B
                    nc.vector.tensor_mul(mFl[:], pkk[:], m_fln)    # -F

                    # ladder: Rm = sum_{j<=15} (-B)^j  (B strict upper in-64)
                    pl = lada[:, 128:384]
                    nc.tensor.matmul(pl[:, 0:128], mNl[:], mNu[:],
                                     start=True, stop=True)        # B^2
                    nc.tensor.matmul(pl[:, 128:256], mNu[:], mNl[:],
                                     start=True, stop=True)        # Nl^2
                    sb2 = recp.tile([128, 256], bf16, name=f"sb2{hc}",
                                    tag="sb2")
                    nc.scalar.copy(sb2[:], pl[:])
                    b2, b2l = sb2[:, 0:128], sb2[:, 128:256]

                    pt1 = lada[:, 384:512]
                    nc.tensor.matmul(pt1[:], mNl[:], b2, start=True,
                                     stop=False)                   # -B^3
                    nc.tensor.matmul(pt1[:], ident, ident, start=False,
                                     stop=False)
                    nc.tensor.matmul(pt1[:], ident, mNu[:], start=False,
                                     stop=False)
                    nc.tensor.matmul(pt1[:], ident, b2, start=False,
                                     stop=True)
                    sT1 = recp.tile([128, 128], bf16, name=f"sT1{hc}",
                                    tag="sT1")
                    nc.scalar.copy(sT1[:], pt1[:])

                    ladd = psw.tile([128, 512], fp32, name=f"ladd{hc}",
                                    tag="lad")
                    # attn P = triu_incl(K Q^T) -- emitted first in this bank
                    pkq = ladd[:, 384:512]
                    for i in range(2):
                        nc.tensor.matmul(pkq[:], KTt[i], QT[i], start=(i == 0),
                                         stop=(i == 1))
                    sPat = recp.tile([128, 128], bf16, name=f"Pat{hc}",
                                     tag="Pat")
                    nc.vector.tensor_mul(sPat[:], pkq[:], m_triuI)
                    pl2 = ladd[:, 0:256]
                    nc.tensor.matmul(pl2[:, 0:128], b2, b2l,
                                     start=True, stop=True)        # Nl^4
                    nc.tensor.matmul(pl2[:, 128:256], b2l, b2,
                                     start=True, stop=True)        # Nu^4
                    sb4 = recp.tile([128, 256], bf16, name=f"sb4{hc}",
                                    tag="sb4")
                    nc.scalar.copy(sb4[:], pl2[:])
                    b4l, b4u = sb4[:, 0:128], sb4[:, 128:256]

                    pr8 = ladd[:, 0:256]
                    nc.tensor.matmul(pr8[:, 0:128], b4l, sT1[:],
                                     start=True, stop=False)       # B^4 T1
                    nc.tensor.matmul(pr8[:, 0:128], ident, sT1[:],
                                     start=False, stop=True)
                    nc.tensor.matmul(pr8[:, 128:256], b4u, b4l,
                                     start=True, stop=True)        # Nl^8
                    sr8 = recp.tile([128, 256], bf16, name=f"sr8{hc}",
                                    tag="sr8")
                    nc.scalar.copy(sr8[:], pr8[:])
                    R8, b8l = sr8[:, 0:128], sr8[:, 128:256]

                    prm = ladd[:, 256:384]
                    nc.tensor.matmul(prm[:], b8l, R8, start=True, stop=False)
                    nc.tensor.matmul(prm[:], ident, R8, start=False, stop=True)
                    sRm = recp.tile([128, 128], bf16, name=f"sRm{hc}",
                                    tag="sRm")
                    nc.vector.tensor_copy(sRm[:], prm[:])

                    # full R^T = Rm (I - G^T);  -G^T = F^T-neg @ Rm
                    pn1 = ladd[:, 384:512]
                    nc.tensor.matmul(pn1[:], mFl[:], sRm[:], start=True,
                                     stop=True)
                    prl = ptr[:, 512:640]
                    nc.tensor.transpose(prl[:], sRm[:], ident)
                    sn1 = recp.tile([128, 256], bf16, name=f"sn1{hc}",
                                    tag="sn1")
                    nc.vector.tensor_copy(sn1[:, 0:128], pn1[:])
                    nc.vector.tensor_copy(sn1[:, 128:256], prl[:])
                    n1, Rl = sn1[:, 0:128], sn1[:, 128:256]

                    pRT = ladd[:, 256:384]
                    nc.tensor.matmul(pRT[:], Rl, n1, start=True, stop=False)
                    nc.tensor.matmul(pRT[:], Rl, ident, start=False, stop=True)
                    sRT = recp.tile([128, 128], bf16, name=f"sRT{hc}",
                                    tag="sRT")
                    nc.vector.tensor_copy(sRT[:], pRT[:])

                    # -M^T = -(K^T R^T)  [Dh, C] as two 128-col slices
                    pmt = ladd[:, 0:256]
                    for i in range(2):
                        nc.tensor.matmul(pmt[:, i * 128:(i + 1) * 128],
                                         kcd[:, i * 128:(i + 1) * 128], sRT[:],
                                         start=True, stop=True)
                    smtn = recp.tile([128, 256], bf16, name=f"smtn{hc}",
                                     tag="smtn")
                    nc.vector.tensor_scalar_mul(smtn[:], pmt[:], -1.0)

                    # U' = R V - M S_prev; upo packs [pu | po]
                    upo = psw.tile([128, 512], fp32, name=f"upo{hc}",
                                   tag="upo")
                    pu = upo[:, 0:256]
                    nc.tensor.matmul(pu[:], sRT[:], vcd, start=True,
                                     stop=False)
                    for i in range(2):
                        nc.tensor.matmul(pu[:], smtn[:, i * 128:(i + 1) * 128],
                                         s_sb[head][:, i * 256:(i + 1) * 256],
                                         start=False, stop=(i == 1))
                    u_sb = recp.tile([128, 256], bf16, name=f"u{hc}", tag="u")
                    nc.vector.tensor_copy(u_sb[:], pu[:])

                    # O = P^T U' + Q S_prev; the u-dependent matmul opens
                    # the group so it cannot start while pu is mid-group in
                    # the same bank.
                    po = upo[:, 256:512]
                    nc.tensor.matmul(po[:], sPat[:], u_sb[:], start=True,
                                     stop=False)
                    for i in range(2):
                        nc.tensor.matmul(po[:], QT[i],
                                         s_sb[head][:, i * 256:(i + 1) * 256],
                                         start=False, stop=(i == 1))

                    # S += K^T U'   (persistent psum accumulator)
                    for i in range(2):
                        # start only on the very first write: PE "start"
                        # flags the whole 2KB bank pending-zero, so a second
                        # start would make the next chunk's accumulate into
                        # the other half overwrite instead of add.
                        nc.tensor.matmul(s_ps[head][:, i * 256:(i + 1) * 256],
                                         kcd[:, i * 128:(i + 1) * 128], u_sb[:],
                                         start=(ch == 0 and i == 0), stop=True,
                                         skip_group_check=True)
                    s_nb = recs.tile([128, 512], bf16, name=f"ssb{hc}",
                                     tag=f"ssb{head}", bufs=2)
                    nc.vector.tensor_copy(s_nb[:, 0:256], s_ps[head][:, 0:256])
                    nc.scalar.copy(s_nb[:, 256:512],
                                   s_ps[head][:, 256:512])
                    s_sb[head] = s_nb

                    # RMSNorm rows of O, transpose into oT[ch]
                    osq = recp.tile([128, 256], bf16, name=f"osq{hc}",
                                    tag="osq")
                    ossq = recp.tile([128, 1], fp32, name=f"ossq{hc}",
                                     tag="ossq")
                    nc.scalar.activation(osq[:], po[:], AF.Square,
                                         accum_out=ossq[:])
                    orsq = recp.tile([128, 1], fp32, name=f"orsq{hc}",
                                     tag="orsq")
                    nc.scalar.activation(orsq[:], ossq[:], AF.Sqrt,
                                         bias=biases[:, 1:2], scale=1.0 / DH)
                    nc.vector.reciprocal(orsq[:], orsq[:])
                    onrm = recp.tile([128, 256], bf16, name=f"onrm{hc}",
                                     tag="onrm")
                    nc.vector.tensor_scalar_mul(onrm[:], po[:], orsq[:])
                    pto = ptr[:, 640:896]
                    for i in range(2):
                        nc.tensor.transpose(pto[:, i * 128:(i + 1) * 128],
                                            onrm[:, i * 128:(i + 1) * 128],
                                            ident)
                    nc.scalar.copy(oTt[:, ct0 * 128:ct0 * 128 + 256], pto[:])

                # ---- phase C for token tile ch (overlaps next chunks) ----
                for half in range(2):
                    pf = psw.tile([128, 512], fp32, name=f"pf{ch}{half}",
                                  tag="upo")
                    for ct in range(CT):
                        nc.tensor.matmul(
                            pf[:], oTt[:, ct * 128:(ct + 1) * 128],
                            wo_s[ct][:, half * 512:(half + 1) * 512],
                            start=(ct == 0), stop=(ct == CT - 1))
                    of = ofp.tile([128, 512], fp32, name=f"of{ch}{half}",
                                  tag="of")
                    nc.scalar.copy(of[:], pf[:])
                    nc.sync.dma_start(
                        out_t[ch][:, half * 512:(half + 1) * 512], of[:])


LP_NP = np.float16  # host-side 16-bit dtype matching the device dtype


def _make_consts():
    ii = np.arange(128)
    blk = ii[:, None] // 64 == ii[None, :] // 64
    low = ii[:, None] > ii[None, :]
    nln = -((low & blk).astype(np.float32))
    nun = nln.T.copy()
    fln = -((low & ~blk).astype(np.float32))
    triuI = (ii[:, None] <= ii[None, :]).astype(np.float32)
    ident = np.eye(128, dtype=np.float32)
    return np.concatenate([ident, nln, nun, fln, triuI],
                          axis=1).astype(LP_NP)


def _get_compiled():
    key = ("nc", SILU_NATIVE)
    if key not in _CACHE:
        _CACHE[key] = _build_bass()
    return _CACHE[key]


def kernel(hidden_states, Wq, Wk, Wv, conv_wq, conv_wk, conv_wv, onorm_w, Wo):
    from concourse.bass_utils import run_bass_kernel_spmd

    hidden_states = np.asarray(hidden_states, np.float32)
    Wq = np.asarray(Wq, np.float32)
    Wk = np.asarray(Wk, np.float32)
    Wv = np.asarray(Wv, np.float32)
    Wo = np.asarray(Wo, np.float32)
    conv_wq = np.asarray(conv_wq, np.float32)
    conv_wk = np.asarray(conv_wk, np.float32)
    conv_wv = np.asarray(conv_wv, np.float32)
    onorm_w = np.asarray(onorm_w, np.float32)

    bf = LP_NP
    consts = _make_consts()
    Wo_eff = (Wo * np.tile(onorm_w, H)[:, None]).astype(bf)  # fold RMS weight

    in_maps = []
    for core in range(NCORES):
        b, g = divmod(core, 2)
        cols = slice(CG * g, CG * (g + 1))
        in_maps.append({
            "xT": np.ascontiguousarray(hidden_states[b].T).astype(bf),
            "wq": np.ascontiguousarray(Wq[:, cols]).astype(bf),
            "wk": np.ascontiguousarray(Wk[:, cols]).astype(bf),
            "wv": np.ascontiguousarray(Wv[:, cols]).astype(bf),
            "wo": np.ascontiguousarray(Wo_eff[cols, :]),
            "cw": np.ascontiguousarray(np.concatenate(
                [conv_wq[cols], conv_wk[cols], conv_wv[cols]], axis=1)),
            "consts": consts,
        })

    nc = _get_compiled()
    res = run_bass_kernel_spmd(nc, in_maps, core_ids=list(range(NCORES)),
                               **_CACHE.get("run_kwargs", {}))
    _CACHE["last_results"] = res
    out = np.zeros((B, T, D), np.float32)
    for core in range(NCORES):
        out[core // 2] += res.results[core]["out"]
    return out


# revision 15
# speedup vs baseline: 1.8240x; 1.0351x over previous
"""DeltaNet forward kernel for 8 Trainium2 NeuronCores.

Problem (hardcoded from the task spec): hidden_states [B=4, T=2048, D=1024],
H=4 heads, Dh=256, causal depthwise conv K=4 + silu on q/k/v projections,
q/k l2-normalized per head (q scaled Dh^-0.5), delta-rule recurrence over T,
per-head RMSNorm, merge heads, out = o @ Wo.

Sharding: data-parallel over (batch, head-group): core c -> batch c//2,
head group c%2 (projection columns [512*(c%2), 512*(c%2)+512)). Each core
computes a partial product against its 512 rows of Wo; the host sums the two
partials per batch (the unshard step for the row-parallel output matmul).

Device algorithm (decoupled WY form, chunk C=128): per chunk and head all
S-independent matrices are precomputed off the critical path:
  KK = K K^T;  R_bd^T = (I+B)^{-1} (B = strict upper of KK in 64-blocks)
  via a power ladder exact to B^15;  R^T = R_bd^T (I - G^T) with
  G = R_bd F (2 blocks of 64 -> G^2 = 0, exact);  M^T = K^T R^T.
The sequential S-chain is only:  U' = R V - M S_prev  (psum-accumulated),
S += K^T U', plus two psum->sbuf copies. O = Q S_prev + triu(K Q^T)^T U',
then per-head RMSNorm and transpose into the output-projection layout.
The Wo projection for token tile tt is emitted right after chunk tt so it
overlaps the recurrence. S accumulates in PSUM f32; matmul operands fp16.
"""

import numpy as np

B, T, D = 4, 2048, 1024
H = 4
DH = D // H          # 256
CONV_K = 4
EPS = 1e-5
NCORES = 8
CG = 512             # columns per core (2 heads)
C = 128              # recurrence chunk length
NCHUNK = T // C      # 16
PAD = 4              # front zero padding on time axis for the causal conv
TOKB = 512           # token block (matmul moving size)
KT = D // 128        # 8 contraction tiles
CT = CG // 128       # 4 column tiles per core
NB = T // TOKB       # 4 token blocks

_CACHE = {}
SILU_NATIVE = True  # CoreSim lacks Silu; set False for simulation runs


def _build_bass():
    import concourse.bass as bass  # noqa: F401
    import concourse.bacc as bacc
    import concourse.mybir as mybir
    import concourse.tile as tile

    dt = mybir.dt
    nc = bacc.Bacc("TRN2", target_bir_lowering=False, debug=False)

    xT = nc.dram_tensor("xT", [D, T], dt.float16, kind="ExternalInput")
    wq = nc.dram_tensor("wq", [D, CG], dt.float16, kind="ExternalInput")
    wk = nc.dram_tensor("wk", [D, CG], dt.float16, kind="ExternalInput")
    wv = nc.dram_tensor("wv", [D, CG], dt.float16, kind="ExternalInput")
    wo = nc.dram_tensor("wo", [CG, D], dt.float16, kind="ExternalInput")
    cw = nc.dram_tensor("cw", [CG, 3 * CONV_K], dt.float32, kind="ExternalInput")
    consts = nc.dram_tensor("consts", [128, 5 * 128], dt.float16,
                            kind="ExternalInput")
    out = nc.dram_tensor("out", [T, D], dt.float32, kind="ExternalOutput")

    with tile.TileContext(nc) as tc:
        _body(nc, tc, mybir, xT, wq, wk, wv, wo, cw, consts, out)

    nc.compile()
    return nc


def _body(nc, tc, mybir, xT, wq, wk, wv, wo, cw, consts, out):
    dt = mybir.dt
    AF = mybir.ActivationFunctionType
    ALU = mybir.AluOpType
    fp32 = dt.float32
    bf16 = dt.float16  # 16-bit working dtype (fp16: 11-bit mantissa)
    NT = T + PAD

    xT_t = xT.ap().rearrange("(n p) t -> n p t", p=128)       # [8,128,T]
    w_t = {"q": wq.ap().rearrange("(n p) c -> n p c", p=128),
           "k": wk.ap().rearrange("(n p) c -> n p c", p=128),
           "v": wv.ap().rearrange("(n p) c -> n p c", p=128)}
    wo_t = wo.ap().rearrange("(n p) c -> n p c", p=128)       # [4,128,D]
    cw_t = cw.ap().rearrange("(n p) c -> n p c", p=128)       # [4,128,12]
    out_t = out.ap().rearrange("(n p) c -> n p c", p=128)     # [16,128,D]

    # ---------- persistent pool (lives for the whole kernel) ----------
    with tc.tile_pool(name="persist", bufs=1) as persist, \
         tc.tile_pool(name="qkvp", bufs=3 * CT) as qkvp, \
         tc.tile_pool(name="otp", bufs=6) as otp, \
         tc.tile_pool(name="wop", bufs=CT) as wop, \
         tc.tile_pool(name="psw", bufs=2, space="PSUM") as psw, \
         tc.tile_pool(name="pss", bufs=1, space="PSUM") as pss:

        cons = persist.tile([128, 5 * 128], bf16, name="cons", tag="cons")
        nc.sync.dma_start(cons[:], consts.ap())
        ident = cons[:, 0:128]          # identity
        m_nln = cons[:, 128:256]        # -(strict lower, in 64-block)
        m_nun = cons[:, 256:384]        # -(strict upper, in 64-block)
        m_fln = cons[:, 384:512]        # -(strict lower, outside 64-blocks)
        m_triuI = cons[:, 512:640]      # i<=j, +1
        ones_col = cons[:, 639:640]     # last col of (i<=j) mask == all ones

        biases = persist.tile([128, 3], dt.float32, name="biases", tag="biases")
        nc.vector.memset(biases[:, 0:1], 1e-6)
        nc.vector.memset(biases[:, 1:2], EPS)
        nc.vector.memset(biases[:, 2:3], 1e-6 * DH)

        wo_s = []
        for ct in range(CT):
            t_ = wop.tile([128, D], bf16, name=f"wo{ct}", tag="wo")
            nc.sync.dma_start(t_[:], wo_t[ct])
            wo_s.append(t_)

        cwt = []
        for ct in range(CT):
            t_ = persist.tile([128, 3 * CONV_K], fp32, name=f"cw{ct}",
                              tag=f"cw{ct}")
            nc.sync.dma_start(t_[:], cw_t[ct])
            cwt.append(t_)

        qh, kh, vh = [], [], []
        for lst, nm in ((qh, "q"), (kh, "k"), (vh, "v")):
            for ct in range(CT):
                lst.append(qkvp.tile([128, T], bf16, name=f"{nm}hat{ct}",
                                     tag="qkv"))

        # ================= phase A: projections + conv + silu + l2norm ====
        with tc.tile_pool(name="xp", bufs=KT) as xp, \
             tc.tile_pool(name="wp", bufs=3 * KT) as wp, \
             tc.tile_pool(name="rawp", bufs=2) as rawp, \
             tc.tile_pool(name="sqp", bufs=4) as sqp, \
             tc.tile_pool(name="stp", bufs=1) as stp, \
             tc.tile_pool(name="bcp", bufs=2) as bcp:

            xt = []
            for kt in range(KT):
                t_ = xp.tile([128, T], bf16, name=f"xt{kt}", tag="xt")
                nc.sync.dma_start(t_[:], xT_t[kt])
                xt.append(t_)
            ws = {}
            for nm in ("q", "k", "v"):
                ws[nm] = []
                for kt in range(KT):
                    t_ = wp.tile([128, CG], bf16, name=f"w{nm}{kt}", tag="w")
                    nc.sync.dma_start(t_[:], w_t[nm][kt])
                    ws[nm].append(t_)

            for ti, (nm, dest) in enumerate((("q", qh), ("k", kh), ("v", vh))):
                sq_tiles = []
                for ct in range(CT):
                    rawt = rawp.tile([128, NT], bf16, name=f"raw{nm}{ct}",
                                     tag="raw")
                    nc.vector.memset(rawt[:, 0:PAD], 0.0)
                    dst = dest[ct]
                    for nb in range(NB):
                        pt = psw.tile([128, TOKB], fp32, name=f"pp{nm}{ct}{nb}",
                                      tag="lad" if nb % 2 else "tr")
                        for kt in range(KT):
                            nc.tensor.matmul(
                                pt[:], ws[nm][kt][:, ct * 128:(ct + 1) * 128],
                                xt[kt][:, nb * TOKB:(nb + 1) * TOKB],
                                start=(kt == 0), stop=(kt == KT - 1))
                        nc.scalar.copy(
                            rawt[:, PAD + nb * TOKB:PAD + (nb + 1) * TOKB],
                            pt[:])
                    # causal depthwise conv along t
                    w0 = cwt[ct][:, ti * CONV_K:ti * CONV_K + 1]
                    nc.vector.tensor_scalar_mul(dst[:], rawt[:, 1:1 + T], w0)
                    for i in range(1, CONV_K):
                        wi = cwt[ct][:, ti * CONV_K + i:ti * CONV_K + i + 1]
                        nc.vector.scalar_tensor_tensor(
                            dst[:], rawt[:, 1 + i:1 + i + T], wi, dst[:],
                            ALU.mult, ALU.add)
                    if SILU_NATIVE:
                        nc.scalar.activation(dst[:], dst[:], AF.Silu)
                    else:
                        sg = rawp.tile([128, T], bf16, name=f"sg{nm}{ct}",
                                       tag="raw")
                        nc.scalar.activation(sg[:], dst[:], AF.Sigmoid)
                        nc.vector.tensor_mul(dst[:], dst[:], sg[:])
                    if ti < 2:
                        sqt = sqp.tile([128, T], bf16, name=f"sq{nm}{ct}",
                                       tag="sq")
                        nc.scalar.activation(sqt[:], dst[:], AF.Square)
                        sq_tiles.append(sqt)
                if ti < 2:
                    # per-head l2norm: sumsq rows via ones-matmul, broadcast
                    # to 128 partitions, rsq = scale/sqrt(ss + 1e-6), apply.
                    for head in range(2):
                        bcf = bcp.tile([128, T], fp32, name=f"bcf{nm}{head}",
                                       tag="bcf")
                        for nb in range(NB):
                            prow = psw.tile([1, TOKB], fp32,
                                            name=f"pr{nm}{head}{nb}",
                                            tag="upo")
                            for cth in range(2):
                                nc.tensor.matmul(
                                    prow[:], ones_col,
                                    sq_tiles[head * 2 + cth][
                                        :, nb * TOKB:(nb + 1) * TOKB],
                                    start=(cth == 0), stop=(cth == 1))
                            rowb = stp.tile([1, TOKB], fp32,
                                            name=f"rb{nm}{head}{nb}",
                                            tag="rowb", bufs=3)
                            nc.scalar.copy(rowb[:], prow[:])
                            nc.gpsimd.partition_broadcast(
                                bcf[:, nb * TOKB:(nb + 1) * TOKB], rowb[:])
                        if ti == 0:
                            # fold Dh^-0.5: 1/(16 sqrt(ss+eps)) =
                            # 1/sqrt(256 ss + 256 eps)
                            nc.scalar.activation(bcf[:], bcf[:], AF.Sqrt,
                                                 bias=biases[:, 2:3],
                                                 scale=float(DH))
                        else:
                            nc.scalar.activation(bcf[:], bcf[:], AF.Sqrt,
                                                 bias=biases[:, 0:1])
                        nc.vector.reciprocal(bcf[:], bcf[:])
                        bcb = bcp.tile([128, T], bf16, name=f"bcb{nm}{head}",
                                       tag="bcb")
                        nc.gpsimd.tensor_copy(bcb[:], bcf[:])
                        for cth in range(2):
                            ct = head * 2 + cth
                            eng = nc.vector if cth else nc.gpsimd
                            eng.tensor_mul(dest[ct][:], dest[ct][:], bcb[:])

        # ================= phase B: delta-rule recurrence + phase C =======
        with tc.tile_pool(name="recp", bufs=4) as recp, \
             tc.tile_pool(name="recs", bufs=1) as recs, \
             tc.tile_pool(name="ofp", bufs=3) as ofp:
            s_ps, s_sb = {}, {}
            for head in range(2):
                s_ps[head] = pss.tile([128, 512], fp32, name=f"sps{head}",
                                      tag=f"sps{head}")
                s_sb[head] = recs.tile([128, 512], bf16, name=f"ssb{head}i",
                                       tag=f"ssb{head}", bufs=2)
                nc.vector.memset(s_sb[head][:], 0.0)

            for ch in range(NCHUNK):
                t0 = ch * C
                oTt = otp.tile([128, 512], bf16, name=f"oT{ch}", tag="oT")
                for head in range(2):
                    ct0 = head * 2
                    QT = [qh[ct0][:, t0:t0 + C], qh[ct0 + 1][:, t0:t0 + C]]
                    KTt = [kh[ct0][:, t0:t0 + C], kh[ct0 + 1][:, t0:t0 + C]]
                    VT = [vh[ct0][:, t0:t0 + C], vh[ct0 + 1][:, t0:t0 + C]]
                    hc = f"{head}_{ch}"

                    # K, V in [C, Dh] via PE transpose into one psum bank
                    # tr bank packs [pkv 0:512 | prl 512:640 | pto 640:896]
                    ptr = psw.tile([128, 896], bf16, name=f"ptr{hc}", tag="tr")
                    pkv = ptr[:, 0:512]
                    for i in range(2):
                        nc.tensor.transpose(pkv[:, i * 128:(i + 1) * 128],
                                            KTt[i], ident)
                        nc.tensor.transpose(pkv[:, 256 + i * 128:384 + i * 128],
                                            VT[i], ident)
                    kvcd = recp.tile([128, 512], bf16, name=f"kvcd{hc}",
                                     tag="kvcd")
                    nc.scalar.copy(kvcd[:], pkv[:])
                    kcd = kvcd[:, 0:256]
                    vcd = kvcd[:, 256:512]

                    # KK^T and masked pieces; lada packs [pkk | b2,b2l | T1]
                    lada = psw.tile([128, 512], fp32, name=f"lada{hc}",
                                    tag="lad")
                    pkk = lada[:, 0:128]
                    for i in range(2):
                        nc.tensor.matmul(pkk[:], KTt[i], KTt[i], start=(i == 0),
                                         stop=(i == 1))
                    mNl = recp.tile([128, 128], bf16, name=f"mNl{hc}",
                                    tag="mNl")
                    mNu = recp.tile([128, 128], bf16, name=f"mNu{hc}",
                                    tag="mNu")
                    mFl = recp.tile([128, 128], bf16, name=f"mFl{hc}",
                                    tag="mFl")
                    nc.vector.tensor_mul(mNl[:], pkk[:], m_nln)    # -B^T
                    nc.vector.tensor_mul(mNu[:], pkk[:], m_nun)    # -B
                    nc.vector.tensor_mul(mFl[:], pkk[:], m_fln)    # -F

                    # ladder: Rm = sum_{j<=15} (-B)^j  (B strict upper in-64)
                    pl = lada[:, 128:384]
                    nc.tensor.matmul(pl[:, 0:128], mNl[:], mNu[:],
                                     start=True, stop=True)        # B^2
                    nc.tensor.matmul(pl[:, 128:256], mNu[:], mNl[:],
                                     start=True, stop=True)        # Nl^2
                    sb2 = recp.tile([128, 256], bf16, name=f"sb2{hc}",
                                    tag="sb2")
                    nc.scalar.copy(sb2[:], pl[:])
                    b2, b2l = sb2[:, 0:128], sb2[:, 128:256]

                    pt1 = lada[:, 384:512]
                    nc.tensor.matmul(pt1[:], mNl[:], b2, start=True,
                                     stop=False)                   # -B^3
                    nc.tensor.matmul(pt1[:], ident, ident, start=False,
                                     stop=False)
                    nc.tensor.matmul(pt1[:], ident, mNu[:], start=False,
                                     stop=False)
                    nc.tensor.matmul(pt1[:], ident, b2, start=False,
                                     stop=True)
                    sT1 = recp.tile([128, 128], bf16, name=f"sT1{hc}",
                                    tag="sT1")
                    nc.scalar.copy(sT1[:], pt1[:])

                    ladd = psw.tile([128, 512], fp32, name=f"ladd{hc}",
                                    tag="lad")
                    # attn P = triu_incl(K Q^T) -- emitted first in this bank
                    pkq = ladd[:, 384:512]
                    for i in range(2):
                        nc.tensor.matmul(pkq[:], KTt[i], QT[i], start=(i == 0),
                                         stop=(i == 1))
                    sPat = recp.tile([128, 128], bf16, name=f"Pat{hc}",
                                     tag="Pat")
                    nc.vector.tensor_mul(sPat[:], pkq[:], m_triuI)
                    pl2 = ladd[:, 0:256]
                    nc.tensor.matmul(pl2[:, 0:128], b2, b2l,
                                     start=True, stop=True)        # Nl^4
                    nc.tensor.matmul(pl2[:, 128:256], b2l, b2,
                                     start=True, stop=True)        # Nu^4
                    sb4 = recp.tile([128, 256], bf16, name=f"sb4{hc}",
                                    tag="sb4")
                    nc.scalar.copy(sb4[:], pl2[:])
                    b4l, b4u = sb4[:, 0:128], sb4[:, 128:256]

                    pr8 = ladd[:, 0:256]
                    nc.tensor.matmul(pr8[:, 0:128], b4l, sT1[:],
                                     start=True, stop=False)       # B^4 T1
                    nc.tensor.matmul(pr8[:, 0:128], ident, sT1[:],
                                     start=False, stop=True)
                    nc.tensor.matmul(pr8[:, 128:256], b4u, b4l,
                                     start=True, stop=True)        # Nl^8
                    sr8 = recp.tile([128, 256], bf16, name=f"sr8{hc}",
                                    tag="sr8")
                    nc.scalar.copy(sr8[:], pr8[:])
                    R8, b8l = sr8[:, 0:128], sr8[:, 128:256]

                    prm = ladd[:, 256:384]
                    nc.tensor.matmul(prm[:], b8l, R8, start=True, stop=False)
                    nc.tensor.matmul(prm[:], ident, R8, start=False, stop=True)
                    sRm = recp.tile([128, 128], bf16, name=f"sRm{hc}",
                                    tag="sRm")
                    nc.vector.tensor_copy(sRm[:], prm[:])

                    # full R^T = Rm (I - G^T);  -G^T = F^T-neg @ Rm
                    pn1 = ladd[:, 384:512]
                    nc.tensor.matmul(pn1[:], mFl[:], sRm[:], start=True,
                                     stop=True)
                    prl = ptr[:, 512:640]
                    nc.tensor.transpose(prl[:], sRm[:], ident)
                    sn1 = recp.tile([128, 256], bf16, name=f"sn1{hc}",
                                    tag="sn1")
                    nc.vector.tensor_copy(sn1[:, 0:128], pn1[:])
                    nc.vector.tensor_copy(sn1[:, 128:256], prl[:])
                    n1, Rl = sn1[:, 0:128], sn1[:, 128:256]

                    pRT = ladd[:, 256:384]
                    nc.tensor.matmul(pRT[:], Rl, n1, start=True, stop=False)
                    nc.tensor.matmul(pRT[:], Rl, ident, start=False, stop=True)
                    sRT = recp.tile([128, 128], bf16, name=f"sRT{hc}",
                                    tag="sRT")
                    nc.vector.tensor_copy(sRT[:], pRT[:])

                    # -M^T = -(K^T R^T)  [Dh, C] as two 128-col slices
                    pmt = ladd[:, 0:256]
                    for i in range(2):
                        nc.tensor.matmul(pmt[:, i * 128:(i + 1) * 128],
                                         kcd[:, i * 128:(i + 1) * 128], sRT[:],
                                         start=True, stop=True)
                    smtn = recp.tile([128, 256], bf16, name=f"smtn{hc}",
                                     tag="smtn")
                    nc.vector.tensor_scalar_mul(smtn[:], pmt[:], -1.0)

                    # U' = R V - M S_prev; upo packs [pu | po]
                    upo = psw.tile([128, 512], fp32, name=f"upo{hc}",
                                   tag="upo")
                    pu = upo[:, 0:256]
                    nc.tensor.matmul(pu[:], sRT[:], vcd, start=True,
                                     stop=False)
                    for i in range(2):
                        nc.tensor.matmul(pu[:], smtn[:, i * 128:(i + 1) * 128],
                                         s_sb[head][:, i * 256:(i + 1) * 256],
                                         start=False, stop=(i == 1))
                    u_sb = recp.tile([128, 256], bf16, name=f"u{hc}", tag="u")
                    nc.vector.tensor_copy(u_sb[:], pu[:])

                    # O = P^T U' + Q S_prev; the u-dependent matmul opens
                    # the group so it cannot start while pu is mid-group in
                    # the same bank.
                    po = upo[:, 256:512]
                    nc.tensor.matmul(po[:], sPat[:], u_sb[:], start=True,
                                     stop=False)
                    for i in range(2):
                        nc.tensor.matmul(po[:], QT[i],
                                         s_sb[head][:, i * 256:(i + 1) * 256],
                                         start=False, stop=(i == 1))

                    # S += K^T U'   (persistent psum accumulator)
                    for i in range(2):
                        # start only on the very first write: PE "start"
                        # flags the whole 2KB bank pending-zero, so a second
                        # start would make the next chunk's accumulate into
                        # the other half overwrite instead of add.
                        nc.tensor.matmul(s_ps[head][:, i * 256:(i + 1) * 256],
                                         kcd[:, i * 128:(i + 1) * 128], u_sb[:],
                                         start=(ch == 0 and i == 0), stop=True,
                                         skip_group_check=True)
                    s_nb = recs.tile([128, 512], bf16, name=f"ssb{hc}",
                                     tag=f"ssb{head}", bufs=2)
                    nc.vector.tensor_copy(s_nb[:, 0:256], s_ps[head][:, 0:256])
                    nc.scalar.copy(s_nb[:, 256:512],
                                   s_ps[head][:, 256:512])
                    s_sb[head] = s_nb

                    # RMSNorm rows of O, transpose into oT[ch]
                    osq = recp.tile([128, 256], bf16, name=f"osq{hc}",
                                    tag="osq")
                    ossq = recp.tile([128, 1], fp32, name=f"ossq{hc}",
                                     tag="ossq")
                    nc.scalar.activation(osq[:], po[:], AF.Square,
                                         accum_out=ossq[:])
                    orsq = recp.tile([128, 1], fp32, name=f"orsq{hc}",
                                     tag="orsq")
                    nc.scalar.activation(orsq[:], ossq[:], AF.Sqrt,
                                         bias=biases[:, 1:2], scale=1.0 / DH)
                    nc.vector.reciprocal(orsq[:], orsq[:])
                    onrm = recp.tile([128, 256], bf16, name=f"onrm{hc}",
                                     tag="onrm")
                    nc.vector.tensor_scalar_mul(onrm[:], po[:], orsq[:])
                    pto = ptr[:, 640:896]
                    for i in range(2):
                        nc.tensor.transpose(pto[:, i * 128:(i + 1) * 128],
                                            onrm[:, i * 128:(i + 1) * 128],
                                            ident)
                    nc.scalar.copy(oTt[:, ct0 * 128:ct0 * 128 + 256], pto[:])

                # ---- phase C for token tile ch (overlaps next chunks) ----
                for half in range(2):
                    pf = psw.tile([128, 512], fp32, name=f"pf{ch}{half}",
                                  tag="upo")
                    for ct in range(CT):
                        nc.tensor.matmul(
                            pf[:], oTt[:, ct * 128:(ct + 1) * 128],
                            wo_s[ct][:, half * 512:(half + 1) * 512],
                            start=(ct == 0), stop=(ct == CT - 1))
                    of = ofp.tile([128, 512], fp32, name=f"of{ch}{half}",
                                  tag="of")
                    nc.scalar.copy(of[:], pf[:])
                    nc.sync.dma_start(
                        out_t[ch][:, half * 512:(half + 1) * 512], of[:])


LP_NP = np.float16  # host-side 16-bit dtype matching the device dtype


def _make_consts():
    ii = np.arange(128)
    blk = ii[:, None] // 64 == ii[None, :] // 64
    low = ii[:, None] > ii[None, :]
    nln = -((low & blk).astype(np.float32))
    nun = nln.T.copy()
    fln = -((low & ~blk).astype(np.float32))
    triuI = (ii[:, None] <= ii[None, :]).astype(np.float32)
    ident = np.eye(128, dtype=np.float32)
    return np.concatenate([ident, nln, nun, fln, triuI],
                          axis=1).astype(LP_NP)


def _get_compiled():
    key = ("nc", SILU_NATIVE)
    if key not in _CACHE:
        _CACHE[key] = _build_bass()
    return _CACHE[key]


def kernel(hidden_states, Wq, Wk, Wv, conv_wq, conv_wk, conv_wv, onorm_w, Wo):
    from concourse.bass_utils import run_bass_kernel_spmd

    hidden_states = np.asarray(hidden_states, np.float32)
    Wq = np.asarray(Wq, np.float32)
    Wk = np.asarray(Wk, np.float32)
    Wv = np.asarray(Wv, np.float32)
    Wo = np.asarray(Wo, np.float32)
    conv_wq = np.asarray(conv_wq, np.float32)
    conv_wk = np.asarray(conv_wk, np.float32)
    conv_wv = np.asarray(conv_wv, np.float32)
    onorm_w = np.asarray(onorm_w, np.float32)

    bf = LP_NP
    consts = _make_consts()
    Wo_eff = (Wo * np.tile(onorm_w, H)[:, None]).astype(bf)  # fold RMS weight

    in_maps = []
    for core in range(NCORES):
        b, g = divmod(core, 2)
        cols = slice(CG * g, CG * (g + 1))
        in_maps.append({
            "xT": np.ascontiguousarray(hidden_states[b].T).astype(bf),
            "wq": np.ascontiguousarray(Wq[:, cols]).astype(bf),
            "wk": np.ascontiguousarray(Wk[:, cols]).astype(bf),
            "wv": np.ascontiguousarray(Wv[:, cols]).astype(bf),
            "wo": np.ascontiguousarray(Wo_eff[cols, :]),
            "cw": np.ascontiguousarray(np.concatenate(
                [conv_wq[cols], conv_wk[cols], conv_wv[cols]], axis=1)),
            "consts": consts,
        })

    nc = _get_compiled()
    res = run_bass_kernel_spmd(nc, in_maps, core_ids=list(range(NCORES)),
                               **_CACHE.get("run_kwargs", {}))
    _CACHE["last_results"] = res
    out = np.zeros((B, T, D), np.float32)
    for core in range(NCORES):
        out[core // 2] += res.results[core]["out"]
    return out


# revision 17
# speedup vs baseline: 1.8374x; 1.0074x over previous
"""DeltaNet forward kernel for 8 Trainium2 NeuronCores.

Problem (hardcoded from the task spec): hidden_states [B=4, T=2048, D=1024],
H=4 heads, Dh=256, causal depthwise conv K=4 + silu on q/k/v projections,
q/k l2-normalized per head (q scaled Dh^-0.5), delta-rule recurrence over T,
per-head RMSNorm, merge heads, out = o @ Wo.

Sharding: data-parallel over (batch, head-group): core c -> batch c//2,
head group c%2 (projection columns [512*(c%2), 512*(c%2)+512)). Each core
computes a partial product against its 512 rows of Wo; the host sums the two
partials per batch (the unshard step for the row-parallel output matmul).

Device algorithm (decoupled WY form, chunk C=128): per chunk and head all
S-independent matrices are precomputed off the critical path:
  KK = K K^T;  R_bd^T = (I+B)^{-1} (B = strict upper of KK in 64-blocks)
  via a power ladder exact to B^15;  R^T = R_bd^T (I - G^T) with
  G = R_bd F (2 blocks of 64 -> G^2 = 0, exact);  M^T = K^T R^T.
The sequential S-chain is only:  U' = R V - M S_prev  (psum-accumulated),
S += K^T U', plus two psum->sbuf copies. O = Q S_prev + triu(K Q^T)^T U',
then per-head RMSNorm and transpose into the output-projection layout.
The Wo projection for token tile tt is emitted right after chunk tt so it
overlaps the recurrence. S accumulates in PSUM f32; matmul operands fp16.
"""

import numpy as np

B, T, D = 4, 2048, 1024
H = 4
DH = D // H          # 256
CONV_K = 4
EPS = 1e-5
NCORES = 8
CG = 512             # columns per core (2 heads)
C = 128              # recurrence chunk length
NCHUNK = T // C      # 16
PAD = 4              # front zero padding on time axis for the causal conv
TOKB = 512           # token block (matmul moving size)
KT = D // 128        # 8 contraction tiles
CT = CG // 128       # 4 column tiles per core
NB = T // TOKB       # 4 token blocks

_CACHE = {}
SILU_NATIVE = True  # CoreSim lacks Silu; set False for simulation runs


def _build_bass():
    import concourse.bass as bass  # noqa: F401
    import concourse.bacc as bacc
    import concourse.mybir as mybir
    import concourse.tile as tile

    dt = mybir.dt
    nc = bacc.Bacc("TRN2", target_bir_lowering=False, debug=False)

    xT = nc.dram_tensor("xT", [D, T], dt.float16, kind="ExternalInput")
    wq = nc.dram_tensor("wq", [D, CG], dt.float16, kind="ExternalInput")
    wk = nc.dram_tensor("wk", [D, CG], dt.float16, kind="ExternalInput")
    wv = nc.dram_tensor("wv", [D, CG], dt.float16, kind="ExternalInput")
    wo = nc.dram_tensor("wo", [CG, D], dt.float16, kind="ExternalInput")
    cw = nc.dram_tensor("cw", [CG, 3 * CONV_K], dt.float32, kind="ExternalInput")
    consts = nc.dram_tensor("consts", [128, 5 * 128], dt.float16,
                            kind="ExternalInput")
    out = nc.dram_tensor("out", [T, D], dt.float32, kind="ExternalOutput")

    with tile.TileContext(nc) as tc:
        _body(nc, tc, mybir, xT, wq, wk, wv, wo, cw, consts, out)

    nc.compile()
    return nc


def _body(nc, tc, mybir, xT, wq, wk, wv, wo, cw, consts, out):
    dt = mybir.dt
    AF = mybir.ActivationFunctionType
    ALU = mybir.AluOpType
    fp32 = dt.float32
    bf16 = dt.float16  # 16-bit working dtype (fp16: 11-bit mantissa)
    NT = T + PAD

    xT_t = xT.ap().rearrange("(n p) t -> n p t", p=128)       # [8,128,T]
    w_t = {"q": wq.ap().rearrange("(n p) c -> n p c", p=128),
           "k": wk.ap().rearrange("(n p) c -> n p c", p=128),
           "v": wv.ap().rearrange("(n p) c -> n p c", p=128)}
    wo_t = wo.ap().rearrange("(n p) c -> n p c", p=128)       # [4,128,D]
    cw_t = cw.ap().rearrange("(n p) c -> n p c", p=128)       # [4,128,12]
    out_t = out.ap().rearrange("(n p) c -> n p c", p=128)     # [16,128,D]

    # ---------- persistent pool (lives for the whole kernel) ----------
    with tc.tile_pool(name="persist", bufs=1) as persist, \
         tc.tile_pool(name="qkvp", bufs=3 * CT) as qkvp, \
         tc.tile_pool(name="otp", bufs=6) as otp, \
         tc.tile_pool(name="wop", bufs=CT) as wop, \
         tc.tile_pool(name="psw", bufs=2, space="PSUM") as psw, \
         tc.tile_pool(name="pss", bufs=1, space="PSUM") as pss:

        cons = persist.tile([128, 5 * 128], bf16, name="cons", tag="cons")
        nc.sync.dma_start(cons[:], consts.ap())
        ident = cons[:, 0:128]          # identity
        m_nln = cons[:, 128:256]        # -(strict lower, in 64-block)
        m_nun = cons[:, 256:384]        # -(strict upper, in 64-block)
        m_fln = cons[:, 384:512]        # -(strict lower, outside 64-blocks)
        m_triuI = cons[:, 512:640]      # i<=j, +1
        ones_col = cons[:, 639:640]     # last col of (i<=j) mask == all ones

        biases = persist.tile([128, 3], dt.float32, name="biases", tag="biases")
        nc.vector.memset(biases[:, 0:1], 1e-6)
        nc.vector.memset(biases[:, 1:2], EPS)
        nc.vector.memset(biases[:, 2:3], 1e-6 * DH)

        wo_s = []
        for ct in range(CT):
            t_ = wop.tile([128, D], bf16, name=f"wo{ct}", tag="wo")
            nc.sync.dma_start(t_[:], wo_t[ct])
            wo_s.append(t_)

        cwt = []
        for ct in range(CT):
            t_ = persist.tile([128, 3 * CONV_K], fp32, name=f"cw{ct}",
                              tag=f"cw{ct}")
            nc.sync.dma_start(t_[:], cw_t[ct])
            cwt.append(t_)

        qh, kh, vh = [], [], []
        for lst, nm in ((qh, "q"), (kh, "k"), (vh, "v")):
            for ct in range(CT):
                lst.append(qkvp.tile([128, T], bf16, name=f"{nm}hat{ct}",
                                     tag="qkv"))

        # ================= phase A: projections + conv + silu + l2norm ====
        with tc.tile_pool(name="xp", bufs=KT) as xp, \
             tc.tile_pool(name="wp", bufs=3 * KT) as wp, \
             tc.tile_pool(name="rawp", bufs=2) as rawp, \
             tc.tile_pool(name="sqp", bufs=4) as sqp, \
             tc.tile_pool(name="stp", bufs=1) as stp, \
             tc.tile_pool(name="bcp", bufs=2) as bcp:

            xt = []
            for kt in range(KT):
                t_ = xp.tile([128, T], bf16, name=f"xt{kt}", tag="xt")
                nc.sync.dma_start(t_[:], xT_t[kt])
                xt.append(t_)
            ws = {}
            for nm in ("q", "k", "v"):
                ws[nm] = []
                for kt in range(KT):
                    t_ = wp.tile([128, CG], bf16, name=f"w{nm}{kt}", tag="w")
                    nc.sync.dma_start(t_[:], w_t[nm][kt])
                    ws[nm].append(t_)

            for ti, (nm, dest) in enumerate((("q", qh), ("k", kh), ("v", vh))):
                sq_tiles = []
                for ct in range(CT):
                    rawt = rawp.tile([128, NT], bf16, name=f"raw{nm}{ct}",
                                     tag="raw")
                    nc.vector.memset(rawt[:, 0:PAD], 0.0)
                    dst = dest[ct]
                    for nb in range(NB):
                        pt = psw.tile([128, TOKB], fp32, name=f"pp{nm}{ct}{nb}",
                                      tag="lad" if nb % 2 else "tr")
                        for kt in range(KT):
                            nc.tensor.matmul(
                                pt[:], ws[nm][kt][:, ct * 128:(ct + 1) * 128],
                                xt[kt][:, nb * TOKB:(nb + 1) * TOKB],
                                start=(kt == 0), stop=(kt == KT - 1))
                        nc.scalar.copy(
                            rawt[:, PAD + nb * TOKB:PAD + (nb + 1) * TOKB],
                            pt[:])
                    # causal depthwise conv along t
                    w0 = cwt[ct][:, ti * CONV_K:ti * CONV_K + 1]
                    nc.vector.tensor_scalar_mul(dst[:], rawt[:, 1:1 + T], w0)
                    for i in range(1, CONV_K):
                        wi = cwt[ct][:, ti * CONV_K + i:ti * CONV_K + i + 1]
                        nc.vector.scalar_tensor_tensor(
                            dst[:], rawt[:, 1 + i:1 + i + T], wi, dst[:],
                            ALU.mult, ALU.add)
                    if SILU_NATIVE:
                        nc.scalar.activation(dst[:], dst[:], AF.Silu)
                    else:
                        sg = rawp.tile([128, T], bf16, name=f"sg{nm}{ct}",
                                       tag="raw")
                        nc.scalar.activation(sg[:], dst[:], AF.Sigmoid)
                        nc.vector.tensor_mul(dst[:], dst[:], sg[:])
                    if ti < 2:
                        sqt = sqp.tile([128, T], bf16, name=f"sq{nm}{ct}",
                                       tag="sq")
                        nc.scalar.activation(sqt[:], dst[:], AF.Square)
                        sq_tiles.append(sqt)
                if ti < 2:
                    # per-head l2norm: sumsq rows via ones-matmul, broadcast
                    # to 128 partitions, rsq = scale/sqrt(ss + 1e-6), apply.
                    for head in range(2):
                        bcf = bcp.tile([128, T], fp32, name=f"bcf{nm}{head}",
                                       tag="bcf")
                        for nb in range(NB):
                            prow = psw.tile([1, TOKB], fp32,
                                            name=f"pr{nm}{head}{nb}",
                                            tag="upo")
                            for cth in range(2):
                                nc.tensor.matmul(
                                    prow[:], ones_col,
                                    sq_tiles[head * 2 + cth][
                                        :, nb * TOKB:(nb + 1) * TOKB],
                                    start=(cth == 0), stop=(cth == 1))
                            rowb = stp.tile([1, TOKB], fp32,
                                            name=f"rb{nm}{head}{nb}",
                                            tag="rowb", bufs=3)
                            nc.scalar.copy(rowb[:], prow[:])
                            nc.gpsimd.partition_broadcast(
                                bcf[:, nb * TOKB:(nb + 1) * TOKB], rowb[:])
                        if ti == 0:
                            # fold Dh^-0.5: 1/(16 sqrt(ss+eps)) =
                            # 1/sqrt(256 ss + 256 eps)
                            nc.scalar.activation(bcf[:], bcf[:], AF.Sqrt,
                                                 bias=biases[:, 2:3],
                                                 scale=float(DH))
                        else:
                            nc.scalar.activation(bcf[:], bcf[:], AF.Sqrt,
                                                 bias=biases[:, 0:1])
                        nc.vector.reciprocal(bcf[:], bcf[:])
                        bcb = bcp.tile([128, T], bf16, name=f"bcb{nm}{head}",
                                       tag="bcb")
                        nc.gpsimd.tensor_copy(bcb[:], bcf[:])
                        for cth in range(2):
                            ct = head * 2 + cth
                            eng = nc.vector if cth else nc.gpsimd
                            eng.tensor_mul(dest[ct][:], dest[ct][:], bcb[:])

        # ================= phase B: delta-rule recurrence + phase C =======
        with tc.tile_pool(name="recp", bufs=4) as recp, \
             tc.tile_pool(name="recs", bufs=1) as recs, \
             tc.tile_pool(name="ofp", bufs=3) as ofp:
            s_ps, s_sb = {}, {}
            for head in range(2):
                s_ps[head] = pss.tile([128, 512], fp32, name=f"sps{head}",
                                      tag=f"sps{head}")
                s_sb[head] = recs.tile([128, 512], bf16, name=f"ssb{head}i",
                                       tag=f"ssb{head}", bufs=2)
                nc.vector.memset(s_sb[head][:], 0.0)

            for ch in range(NCHUNK):
                t0 = ch * C
                oTt = otp.tile([128, 512], bf16, name=f"oT{ch}", tag="oT")
                for head in range(2):
                    ct0 = head * 2
                    QT = [qh[ct0][:, t0:t0 + C], qh[ct0 + 1][:, t0:t0 + C]]
                    KTt = [kh[ct0][:, t0:t0 + C], kh[ct0 + 1][:, t0:t0 + C]]
                    VT = [vh[ct0][:, t0:t0 + C], vh[ct0 + 1][:, t0:t0 + C]]
                    hc = f"{head}_{ch}"

                    # K, V in [C, Dh] via PE transpose into one psum bank
                    # tr bank packs [pkv 0:512 | prl 512:640 | pto 640:896]
                    ptr = psw.tile([128, 896], bf16, name=f"ptr{hc}", tag="tr")
                    pkv = ptr[:, 0:512]
                    for i in range(2):
                        nc.tensor.transpose(pkv[:, i * 128:(i + 1) * 128],
                                            KTt[i], ident)
                        nc.tensor.transpose(pkv[:, 256 + i * 128:384 + i * 128],
                                            VT[i], ident)
                    kvcd = recp.tile([128, 512], bf16, name=f"kvcd{hc}",
                                     tag="kvcd")
                    nc.scalar.copy(kvcd[:], pkv[:])
                    kcd = kvcd[:, 0:256]
                    vcd = kvcd[:, 256:512]

                    # KK^T and masked pieces; lada packs [pkk | b2,b2l | T1]
                    lada = psw.tile([128, 512], fp32, name=f"lada{hc}",
                                    tag="lad")
                    pkk = lada[:, 0:128]
                    for i in range(2):
                        nc.tensor.matmul(pkk[:], KTt[i], KTt[i], start=(i == 0),
                                         stop=(i == 1))
                    mNl = recp.tile([128, 128], bf16, name=f"mNl{hc}",
                                    tag="mNl")
                    mNu = recp.tile([128, 128], bf16, name=f"mNu{hc}",
                                    tag="mNu")
                    mFl = recp.tile([128, 128], bf16, name=f"mFl{hc}",
                                    tag="mFl")
                    nc.vector.tensor_mul(mNl[:], pkk[:], m_nln)    # -B^T
                    nc.vector.tensor_mul(mNu[:], pkk[:], m_nun)    # -B
                    nc.vector.tensor_mul(mFl[:], pkk[:], m_fln)    # -F

                    # ladder: Rm = sum_{j<=15} (-B)^j  (B strict upper in-64)
                    pl = lada[:, 128:384]
                    nc.tensor.matmul(pl[:, 0:128], mNl[:], mNu[:],
                                     start=True, stop=True)        # B^2
                    nc.tensor.matmul(pl[:, 128:256], mNu[:], mNl[:],
                                     start=True, stop=True)        # Nl^2
                    sb2 = recp.tile([128, 256], bf16, name=f"sb2{hc}",
                                    tag="sb2")
                    nc.scalar.copy(sb2[:], pl[:])
                    b2, b2l = sb2[:, 0:128], sb2[:, 128:256]

                    pt1 = lada[:, 384:512]
                    nc.tensor.matmul(pt1[:], mNl[:], b2, start=True,
                                     stop=False)                   # -B^3
                    nc.tensor.matmul(pt1[:], ident, ident, start=False,
                                     stop=False)
                    nc.tensor.matmul(pt1[:], ident, mNu[:], start=False,
                                     stop=False)
                    nc.tensor.matmul(pt1[:], ident, b2, start=False,
                                     stop=True)
                    sT1 = recp.tile([128, 128], bf16, name=f"sT1{hc}",
                                    tag="sT1")
                    nc.scalar.copy(sT1[:], pt1[:])

                    ladd = psw.tile([128, 512], fp32, name=f"ladd{hc}",
                                    tag="lad")
                    # attn P = triu_incl(K Q^T) -- emitted first in this bank
                    pkq = ladd[:, 384:512]
                    for i in range(2):
                        nc.tensor.matmul(pkq[:], KTt[i], QT[i], start=(i == 0),
                                         stop=(i == 1))
                    sPat = recp.tile([128, 128], bf16, name=f"Pat{hc}",
                                     tag="Pat")
                    nc.vector.tensor_mul(sPat[:], pkq[:], m_triuI)
                    pl2 = ladd[:, 0:256]
                    nc.tensor.matmul(pl2[:, 0:128], b2, b2l,
                                     start=True, stop=True)        # Nl^4
                    nc.tensor.matmul(pl2[:, 128:256], b2l, b2,
                                     start=True, stop=True)        # Nu^4
                    sb4 = recp.tile([128, 256], bf16, name=f"sb4{hc}",
                                    tag="sb4")
                    nc.scalar.copy(sb4[:], pl2[:])
                    b4l, b4u = sb4[:, 0:128], sb4[:, 128:256]

                    pr8 = ladd[:, 0:256]
                    nc.tensor.matmul(pr8[:, 0:128], b4l, sT1[:],
                                     start=True, stop=False)       # B^4 T1
                    nc.tensor.matmul(pr8[:, 0:128], ident, sT1[:],
                                     start=False, stop=True)
                    nc.tensor.matmul(pr8[:, 128:256], b4u, b4l,
                                     start=True, stop=True)        # Nl^8
                    sr8 = recp.tile([128, 256], bf16, name=f"sr8{hc}",
                                    tag="sr8")
                    nc.scalar.copy(sr8[:], pr8[:])
                    R8, b8l = sr8[:, 0:128], sr8[:, 128:256]

                    prm = ladd[:, 256:384]
                    nc.tensor.matmul(prm[:], b8l, R8, start=True, stop=False)
                    nc.tensor.matmul(prm[:], ident, R8, start=False, stop=True)
                    sRm = recp.tile([128, 128], bf16, name=f"sRm{hc}",
                                    tag="sRm")
                    nc.vector.tensor_copy(sRm[:], prm[:])

                    # full R^T = Rm (I - G^T);  -G^T = F^T-neg @ Rm
                    pn1 = ladd[:, 384:512]
                    nc.tensor.matmul(pn1[:], mFl[:], sRm[:], start=True,
                                     stop=True)
                    prl = ptr[:, 512:640]
                    nc.tensor.transpose(prl[:], sRm[:], ident)
                    sn1 = recp.tile([128, 256], bf16, name=f"sn1{hc}",
                                    tag="sn1")
                    nc.vector.tensor_copy(sn1[:, 0:128], pn1[:])
                    nc.vector.tensor_copy(sn1[:, 128:256], prl[:])
                    n1, Rl = sn1[:, 0:128], sn1[:, 128:256]

                    pRT = ladd[:, 256:384]
                    nc.tensor.matmul(pRT[:], Rl, n1, start=True, stop=False)
                    nc.tensor.matmul(pRT[:], Rl, ident, start=False, stop=True)
                    sRT = recp.tile([128, 128], bf16, name=f"sRT{hc}",
                                    tag="sRT")
                    nc.vector.tensor_copy(sRT[:], pRT[:])

                    # -M^T = -(K^T R^T)  [Dh, C] as two 128-col slices
                    pmt = ladd[:, 0:256]
                    for i in range(2):
                        nc.tensor.matmul(pmt[:, i * 128:(i + 1) * 128],
                                         kcd[:, i * 128:(i + 1) * 128], sRT[:],
                                         start=True, stop=True)
                    smtn = recp.tile([128, 256], bf16, name=f"smtn{hc}",
                                     tag="smtn")
                    nc.vector.tensor_scalar_mul(smtn[:], pmt[:], -1.0)

                    # U' = R V - M S_prev; upo packs [pu | po]
                    upo = psw.tile([128, 512], fp32, name=f"upo{hc}",
                                   tag="upo")
                    pu = upo[:, 0:256]
                    nc.tensor.matmul(pu[:], sRT[:], vcd, start=True,
                                     stop=False)
                    for i in range(2):
                        nc.tensor.matmul(pu[:], smtn[:, i * 128:(i + 1) * 128],
                                         s_sb[head][:, i * 256:(i + 1) * 256],
                                         start=False, stop=(i == 1))
                    u_sb = recp.tile([128, 256], bf16, name=f"u{hc}", tag="u")
                    nc.vector.tensor_copy(u_sb[:], pu[:])

                    # O = P^T U' + Q S_prev; the u-dependent matmul opens
                    # the group so it cannot start while pu is mid-group in
                    # the same bank.
                    po = upo[:, 256:512]
                    nc.tensor.matmul(po[:], sPat[:], u_sb[:], start=True,
                                     stop=False)
                    for i in range(2):
                        nc.tensor.matmul(po[:], QT[i],
                                         s_sb[head][:, i * 256:(i + 1) * 256],
                                         start=False, stop=(i == 1))

                    # S += K^T U'   (persistent psum accumulator)
                    for i in range(2):
                        # start only on the very first write: PE "start"
                        # flags the whole 2KB bank pending-zero, so a second
                        # start would make the next chunk's accumulate into
                        # the other half overwrite instead of add.
                        nc.tensor.matmul(s_ps[head][:, i * 256:(i + 1) * 256],
                                         kcd[:, i * 128:(i + 1) * 128], u_sb[:],
                                         start=(ch == 0 and i == 0), stop=True,
                                         skip_group_check=True)
                    s_nb = recs.tile([128, 512], bf16, name=f"ssb{hc}",
                                     tag=f"ssb{head}", bufs=2)
                    nc.vector.tensor_copy(s_nb[:, 0:256], s_ps[head][:, 0:256])
                    nc.scalar.copy(s_nb[:, 256:512],
                                   s_ps[head][:, 256:512])
                    s_sb[head] = s_nb

                    # RMSNorm rows of O, transpose into oT[ch]
                    osq = recp.tile([128, 256], bf16, name=f"osq{hc}",
                                    tag="osq")
                    ossq = recp.tile([128, 1], fp32, name=f"ossq{hc}",
                                     tag="ossq")
                    nc.scalar.activation(osq[:], po[:], AF.Square,
                                         accum_out=ossq[:])
                    orsq = recp.tile([128, 1], fp32, name=f"orsq{hc}",
                                     tag="orsq")
                    nc.scalar.activation(orsq[:], ossq[:], AF.Sqrt,
                                         bias=biases[:, 1:2], scale=1.0 / DH)
                    nc.vector.reciprocal(orsq[:], orsq[:])
                    onrm = recp.tile([128, 256], bf16, name=f"onrm{hc}",
                                     tag="onrm")
                    nc.vector.tensor_scalar_mul(onrm[:], po[:], orsq[:])
                    pto = ptr[:, 640:896]
                    for i in range(2):
                        nc.tensor.transpose(pto[:, i * 128:(i + 1) * 128],
                                            onrm[:, i * 128:(i + 1) * 128],
                                            ident)
                    nc.scalar.copy(oTt[:, ct0 * 128:ct0 * 128 + 256], pto[:])

                # ---- phase C for token tile ch (overlaps next chunks) ----
                for half in range(2):
                    pf = psw.tile([128, 512], fp32, name=f"pf{ch}{half}",
                                  tag="upo")
                    for ct in range(CT):
                        nc.tensor.matmul(
                            pf[:], oTt[:, ct * 128:(ct + 1) * 128],
                            wo_s[ct][:, half * 512:(half + 1) * 512],
                            start=(ct == 0), stop=(ct == CT - 1))
                    of = ofp.tile([128, 512], fp32, name=f"of{ch}{half}",
                                  tag="of")
                    nc.scalar.copy(of[:], pf[:])
                    nc.sync.dma_start(
                        out_t[ch][:, half * 512:(half + 1) * 512], of[:])


LP_NP = np.float16  # host-side 16-bit dtype matching the device dtype


def _make_consts():
    ii = np.arange(128)
    blk = ii[:, None] // 64 == ii[None, :] // 64
    low = ii[:, None] > ii[None, :]
    nln = -((low & blk).astype(np.float32))
    nun = nln.T.copy()
    fln = -((low & ~blk).astype(np.float32))
    triuI = (ii[:, None] <= ii[None, :]).astype(np.float32)
    ident = np.eye(128, dtype=np.float32)
    return np.concatenate([ident, nln, nun, fln, triuI],
                          axis=1).astype(LP_NP)


def _get_compiled():
    key = ("nc", SILU_NATIVE)
    if key not in _CACHE:
        _CACHE[key] = _build_bass()
    return _CACHE[key]


def kernel(hidden_states, Wq, Wk, Wv, conv_wq, conv_wk, conv_wv, onorm_w, Wo):
    from concourse.bass_utils import run_bass_kernel_spmd

    hidden_states = np.asarray(hidden_states, np.float32)
    Wq = np.asarray(Wq, np.float32)
    Wk = np.asarray(Wk, np.float32)
    Wv = np.asarray(Wv, np.float32)
    Wo = np.asarray(Wo, np.float32)
    conv_wq = np.asarray(conv_wq, np.float32)
    conv_wk = np.asarray(conv_wk, np.float32)
    conv_wv = np.asarray(conv_wv, np.float32)
    onorm_w = np.asarray(onorm_w, np.float32)

    bf = LP_NP
    consts = _make_consts()
    Wo_eff = (Wo * np.tile(onorm_w, H)[:, None]).astype(bf)  # fold RMS weight

    in_maps = []
    for core in range(NCORES):
        b, g = divmod(core, 2)
        cols = slice(CG * g, CG * (g + 1))
        in_maps.append({
            "xT": np.ascontiguousarray(hidden_states[b].T).astype(bf),
            "wq": np.ascontiguousarray(Wq[:, cols]).astype(bf),
            "wk": np.ascontiguousarray(Wk[:, cols]).astype(bf),
            "wv": np.ascontiguousarray(Wv[:, cols]).astype(bf),
            "wo": np.ascontiguousarray(Wo_eff[cols, :]),
            "cw": np.ascontiguousarray(np.concatenate(
                [conv_wq[cols], conv_wk[cols], conv_wv[cols]], axis=1)),
            "consts": consts,
        })

    nc = _get_compiled()
    res = run_bass_kernel_spmd(nc, in_maps, core_ids=list(range(NCORES)),
                               **_CACHE.get("run_kwargs", {}))
    _CACHE["last_results"] = res
    out = np.zeros((B, T, D), np.float32)
    for core in range(NCORES):
        out[core // 2] += res.results[core]["out"]
    return out


# revision 18
# speedup vs baseline: 1.8712x; 1.0184x over previous
"""DeltaNet forward kernel for 8 Trainium2 NeuronCores.

Problem (hardcoded from the task spec): hidden_states [B=4, T=2048, D=1024],
H=4 heads, Dh=256, causal depthwise conv K=4 + silu on q/k/v projections,
q/k l2-normalized per head (q scaled Dh^-0.5), delta-rule recurrence over T,
per-head RMSNorm, merge heads, out = o @ Wo.

Sharding: data-parallel over (batch, head-group): core c -> batch c//2,
head group c%2 (projection columns [512*(c%2), 512*(c%2)+512)). Each core
computes a partial product against its 512 rows of Wo; the host sums the two
partials per batch (the unshard step for the row-parallel output matmul).

Device algorithm (decoupled WY form, chunk C=128): per chunk and head all
S-independent matrices are precomputed off the critical path:
  KK = K K^T;  R_bd^T = (I+B)^{-1} (B = strict upper of KK in 64-blocks)
  via a power ladder exact to B^15;  R^T = R_bd^T (I - G^T) with
  G = R_bd F (2 blocks of 64 -> G^2 = 0, exact);  M^T = K^T R^T.
The sequential S-chain is only:  U' = R V - M S_prev  (psum-accumulated),
S += K^T U', plus two psum->sbuf copies. O = Q S_prev + triu(K Q^T)^T U',
then per-head RMSNorm and transpose into the output-projection layout.
The Wo projection for token tile tt is emitted right after chunk tt so it
overlaps the recurrence. S accumulates in PSUM f32; matmul operands fp16.
"""

import numpy as np

B, T, D = 4, 2048, 1024
H = 4
DH = D // H          # 256
CONV_K = 4
EPS = 1e-5
NCORES = 8
CG = 512             # columns per core (2 heads)
C = 128              # recurrence chunk length
NCHUNK = T // C      # 16
PAD = 4              # front zero padding on time axis for the causal conv
TOKB = 512           # token block (matmul moving size)
KT = D // 128        # 8 contraction tiles
CT = CG // 128       # 4 column tiles per core
NB = T // TOKB       # 4 token blocks

_CACHE = {}
SILU_NATIVE = True  # CoreSim lacks Silu; set False for simulation runs


def _build_bass():
    import concourse.bass as bass  # noqa: F401
    import concourse.bacc as bacc
    import concourse.mybir as mybir
    import concourse.tile as tile

    dt = mybir.dt
    nc = bacc.Bacc("TRN2", target_bir_lowering=False, debug=False)

    xT = nc.dram_tensor("xT", [D, T], dt.float16, kind="ExternalInput")
    wq = nc.dram_tensor("wq", [D, CG], dt.float16, kind="ExternalInput")
    wk = nc.dram_tensor("wk", [D, CG], dt.float16, kind="ExternalInput")
    wv = nc.dram_tensor("wv", [D, CG], dt.float16, kind="ExternalInput")
    wo = nc.dram_tensor("wo", [CG, D], dt.float16, kind="ExternalInput")
    cw = nc.dram_tensor("cw", [CG, 3 * CONV_K], dt.float32, kind="ExternalInput")
    consts = nc.dram_tensor("consts", [128, 5 * 128], dt.float16,
                            kind="ExternalInput")
    out = nc.dram_tensor("out", [T, D], dt.float32, kind="ExternalOutput")

    with tile.TileContext(nc) as tc:
        _body(nc, tc, mybir, xT, wq, wk, wv, wo, cw, consts, out)

    nc.compile()
    return nc


def _body(nc, tc, mybir, xT, wq, wk, wv, wo, cw, consts, out):
    dt = mybir.dt
    AF = mybir.ActivationFunctionType
    ALU = mybir.AluOpType
    fp32 = dt.float32
    bf16 = dt.float16  # 16-bit working dtype (fp16: 11-bit mantissa)
    NT = T + PAD

    xT_t = xT.ap().rearrange("(n p) t -> n p t", p=128)       # [8,128,T]
    w_t = {"q": wq.ap().rearrange("(n p) c -> n p c", p=128),
           "k": wk.ap().rearrange("(n p) c -> n p c", p=128),
           "v": wv.ap().rearrange("(n p) c -> n p c", p=128)}
    wo_t = wo.ap().rearrange("(n p) c -> n p c", p=128)       # [4,128,D]
    cw_t = cw.ap().rearrange("(n p) c -> n p c", p=128)       # [4,128,12]
    out_t = out.ap().rearrange("(n p) c -> n p c", p=128)     # [16,128,D]

    # ---------- persistent pool (lives for the whole kernel) ----------
    with tc.tile_pool(name="persist", bufs=1) as persist, \
         tc.tile_pool(name="qkvp", bufs=3 * CT) as qkvp, \
         tc.tile_pool(name="otp", bufs=6) as otp, \
         tc.tile_pool(name="wop", bufs=CT) as wop, \
         tc.tile_pool(name="psw", bufs=2, space="PSUM") as psw, \
         tc.tile_pool(name="pss", bufs=1, space="PSUM") as pss:

        cons = persist.tile([128, 5 * 128], bf16, name="cons", tag="cons")
        nc.sync.dma_start(cons[:], consts.ap())
        ident = cons[:, 0:128]          # identity
        m_nln = cons[:, 128:256]        # -(strict lower, in 64-block)
        m_nun = cons[:, 256:384]        # -(strict upper, in 64-block)
        m_fln = cons[:, 384:512]        # -(strict lower, outside 64-blocks)
        m_triuI = cons[:, 512:640]      # i<=j, +1
        ones_col = cons[:, 639:640]     # last col of (i<=j) mask == all ones

        biases = persist.tile([128, 3], dt.float32, name="biases", tag="biases")
        nc.vector.memset(biases[:, 0:1], 1e-6)
        nc.vector.memset(biases[:, 1:2], EPS)
        nc.vector.memset(biases[:, 2:3], 1e-6 * DH)

        wo_s = []
        for ct in range(CT):
            t_ = wop.tile([128, D], bf16, name=f"wo{ct}", tag="wo")
            nc.sync.dma_start(t_[:], wo_t[ct])
            wo_s.append(t_)

        cwt = []
        for ct in range(CT):
            t_ = persist.tile([128, 3 * CONV_K], fp32, name=f"cw{ct}",
                              tag=f"cw{ct}")
            nc.sync.dma_start(t_[:], cw_t[ct])
            cwt.append(t_)

        qh, kh, vh = [], [], []
        for lst, nm in ((qh, "q"), (kh, "k"), (vh, "v")):
            for ct in range(CT):
                lst.append(qkvp.tile([128, T], bf16, name=f"{nm}hat{ct}",
                                     tag="qkv"))

        # ================= phase A: projections + conv + silu + l2norm ====
        with tc.tile_pool(name="xp", bufs=KT) as xp, \
             tc.tile_pool(name="wp", bufs=3 * KT) as wp, \
             tc.tile_pool(name="rawp", bufs=2) as rawp, \
             tc.tile_pool(name="sqp", bufs=4) as sqp, \
             tc.tile_pool(name="stp", bufs=1) as stp, \
             tc.tile_pool(name="bcp", bufs=2) as bcp:

            # demand-ordered input DMAs: x arrives per token block, with
            # each name's weights queued right after the block that first
            # needs them, so block-0 projections start ~8us earlier
            xt = [xp.tile([128, T], bf16, name=f"xt{kt}", tag="xt")
                  for kt in range(KT)]
            ws = {nm: [wp.tile([128, CG], bf16, name=f"w{nm}{kt}", tag="w")
                       for kt in range(KT)]
                  for nm in ("q", "k", "v")}
            wsnames = ["q", "k", "v", None]
            for nb in range(NB):
                c0 = nb * TOKB
                for kt in range(KT):
                    nc.sync.dma_start(xt[kt][:, c0:c0 + TOKB],
                                      xT_t[kt][:, c0:c0 + TOKB])
                if wsnames[nb]:
                    for kt in range(KT):
                        nc.sync.dma_start(ws[wsnames[nb]][kt][:],
                                          w_t[wsnames[nb]][kt])

            for ti, (nm, dest) in enumerate((("q", qh), ("k", kh), ("v", vh))):
                sq_tiles = []
                for ct in range(CT):
                    rawt = rawp.tile([128, NT], bf16, name=f"raw{nm}{ct}",
                                     tag="raw")
                    nc.vector.memset(rawt[:, 0:PAD], 0.0)
                    dst = dest[ct]
                    for nb in range(NB):
                        pt = psw.tile([128, TOKB], fp32, name=f"pp{nm}{ct}{nb}",
                                      tag="lad" if nb % 2 else "tr")
                        for kt in range(KT):
                            nc.tensor.matmul(
                                pt[:], ws[nm][kt][:, ct * 128:(ct + 1) * 128],
                                xt[kt][:, nb * TOKB:(nb + 1) * TOKB],
                                start=(kt == 0), stop=(kt == KT - 1))
                        nc.scalar.copy(
                            rawt[:, PAD + nb * TOKB:PAD + (nb + 1) * TOKB],
                            pt[:])
                    # causal depthwise conv along t
                    w0 = cwt[ct][:, ti * CONV_K:ti * CONV_K + 1]
                    nc.vector.tensor_scalar_mul(dst[:], rawt[:, 1:1 + T], w0)
                    for i in range(1, CONV_K):
                        wi = cwt[ct][:, ti * CONV_K + i:ti * CONV_K + i + 1]
                        nc.vector.scalar_tensor_tensor(
                            dst[:], rawt[:, 1 + i:1 + i + T], wi, dst[:],
                            ALU.mult, ALU.add)
                    if SILU_NATIVE:
                        nc.scalar.activation(dst[:], dst[:], AF.Silu)
                    else:
                        sg = rawp.tile([128, T], bf16, name=f"sg{nm}{ct}",
                                       tag="raw")
                        nc.scalar.activation(sg[:], dst[:], AF.Sigmoid)
                        nc.vector.tensor_mul(dst[:], dst[:], sg[:])
                    if ti < 2:
                        sqt = sqp.tile([128, T], bf16, name=f"sq{nm}{ct}",
                                       tag="sq")
                        nc.scalar.activation(sqt[:], dst[:], AF.Square)
                        sq_tiles.append(sqt)
                if ti < 2:
                    # per-head l2norm: sumsq rows via ones-matmul, broadcast
                    # to 128 partitions, rsq = scale/sqrt(ss + 1e-6), apply.
                    for head in range(2):
                        bcf = bcp.tile([128, T], fp32, name=f"bcf{nm}{head}",
                                       tag="bcf")
                        for nb in range(NB):
                            prow = psw.tile([1, TOKB], fp32,
                                            name=f"pr{nm}{head}{nb}",
                                            tag="upo")
                            for cth in range(2):
                                nc.tensor.matmul(
                                    prow[:], ones_col,
                                    sq_tiles[head * 2 + cth][
                                        :, nb * TOKB:(nb + 1) * TOKB],
                                    start=(cth == 0), stop=(cth == 1))
                            rowb = stp.tile([1, TOKB], fp32,
                                            name=f"rb{nm}{head}{nb}",
                                            tag="rowb", bufs=3)
                            nc.scalar.copy(rowb[:], prow[:])
                            nc.gpsimd.partition_broadcast(
                                bcf[:, nb * TOKB:(nb + 1) * TOKB], rowb[:])
                        if ti == 0:
                            # fold Dh^-0.5: 1/(16 sqrt(ss+eps)) =
                            # 1/sqrt(256 ss + 256 eps)
                            nc.scalar.activation(bcf[:], bcf[:], AF.Sqrt,
                                                 bias=biases[:, 2:3],
                                                 scale=float(DH))
                        else:
                            nc.scalar.activation(bcf[:], bcf[:], AF.Sqrt,
                                                 bias=biases[:, 0:1])
                        nc.vector.reciprocal(bcf[:], bcf[:])
                        bcb = bcp.tile([128, T], bf16, name=f"bcb{nm}{head}",
                                       tag="bcb")
                        nc.gpsimd.tensor_copy(bcb[:], bcf[:])
                        for cth in range(2):
                            ct = head * 2 + cth
                            eng = nc.vector if cth else nc.gpsimd
                            eng.tensor_mul(dest[ct][:], dest[ct][:], bcb[:])

        # ================= phase B: delta-rule recurrence + phase C =======
        with tc.tile_pool(name="recp", bufs=4) as recp, \
             tc.tile_pool(name="recs", bufs=1) as recs, \
             tc.tile_pool(name="ofp", bufs=3) as ofp:
            s_ps, s_sb = {}, {}
            for head in range(2):
                s_ps[head] = pss.tile([128, 512], fp32, name=f"sps{head}",
                                      tag=f"sps{head}")
                s_sb[head] = recs.tile([128, 512], bf16, name=f"ssb{head}i",
                                       tag=f"ssb{head}", bufs=2)
                nc.vector.memset(s_sb[head][:], 0.0)

            for ch in range(NCHUNK):
                t0 = ch * C
                oTt = otp.tile([128, 512], bf16, name=f"oT{ch}", tag="oT")
                for head in range(2):
                    ct0 = head * 2
                    QT = [qh[ct0][:, t0:t0 + C], qh[ct0 + 1][:, t0:t0 + C]]
                    KTt = [kh[ct0][:, t0:t0 + C], kh[ct0 + 1][:, t0:t0 + C]]
                    VT = [vh[ct0][:, t0:t0 + C], vh[ct0 + 1][:, t0:t0 + C]]
                    hc = f"{head}_{ch}"

                    # K, V in [C, Dh] via PE transpose into one psum bank
                    # tr bank packs [pkv 0:512 | prl 512:640 | pto 640:896]
                    ptr = psw.tile([128, 896], bf16, name=f"ptr{hc}", tag="tr")
                    pkv = ptr[:, 0:512]
                    for i in range(2):
                        nc.tensor.transpose(pkv[:, i * 128:(i + 1) * 128],
                                            KTt[i], ident)
                        nc.tensor.transpose(pkv[:, 256 + i * 128:384 + i * 128],
                                            VT[i], ident)
                    kvcd = recp.tile([128, 512], bf16, name=f"kvcd{hc}",
                                     tag="kvcd")
                    nc.scalar.copy(kvcd[:], pkv[:])
                    kcd = kvcd[:, 0:256]
                    vcd = kvcd[:, 256:512]

                    # KK^T and masked pieces; lada packs [pkk | b2,b2l | T1]
                    lada = psw.tile([128, 512], fp32, name=f"lada{hc}",
                                    tag="lad")
                    pkk = lada[:, 0:128]
                    for i in range(2):
                        nc.tensor.matmul(pkk[:], KTt[i], KTt[i], start=(i == 0),
                                         stop=(i == 1))
                    mNl = recp.tile([128, 128], bf16, name=f"mNl{hc}",
                                    tag="mNl")
                    mNu = recp.tile([128, 128], bf16, name=f"mNu{hc}",
                                    tag="mNu")
                    mFl = recp.tile([128, 128], bf16, name=f"mFl{hc}",
                                    tag="mFl")
                    nc.vector.tensor_mul(mNl[:], pkk[:], m_nln)    # -B^T
                    nc.vector.tensor_mul(mNu[:], pkk[:], m_nun)    # -B
                    nc.vector.tensor_mul(mFl[:], pkk[:], m_fln)    # -F

                    # ladder: Rm = sum_{j<=15} (-B)^j  (B strict upper in-64)
                    pl = lada[:, 128:384]
                    nc.tensor.matmul(pl[:, 0:128], mNl[:], mNu[:],
                                     start=True, stop=True)        # B^2
                    nc.tensor.matmul(pl[:, 128:256], mNu[:], mNl[:],
                                     start=True, stop=True)        # Nl^2
                    sb2 = recp.tile([128, 256], bf16, name=f"sb2{hc}",
                                    tag="sb2")
                    nc.scalar.copy(sb2[:], pl[:])
                    b2, b2l = sb2[:, 0:128], sb2[:, 128:256]

                    pt1 = lada[:, 384:512]
                    nc.tensor.matmul(pt1[:], mNl[:], b2, start=True,
                                     stop=False)                   # -B^3
                    nc.tensor.matmul(pt1[:], ident, ident, start=False,
                                     stop=False)
                    nc.tensor.matmul(pt1[:], ident, mNu[:], start=False,
                                     stop=False)
                    nc.tensor.matmul(pt1[:], ident, b2, start=False,
                                     stop=True)
                    sT1 = recp.tile([128, 128], bf16, name=f"sT1{hc}",
                                    tag="sT1")
                    nc.scalar.copy(sT1[:], pt1[:])

                    ladd = psw.tile([128, 512], fp32, name=f"ladd{hc}",
                                    tag="lad")
                    # attn P = triu_incl(K Q^T) -- emitted first in this bank
                    pkq = ladd[:, 384:512]
                    for i in range(2):
                        nc.tensor.matmul(pkq[:], KTt[i], QT[i], start=(i == 0),
                                         stop=(i == 1))
                    sPat = recp.tile([128, 128], bf16, name=f"Pat{hc}",
                                     tag="Pat")
                    nc.vector.tensor_mul(sPat[:], pkq[:], m_triuI)
                    pl2 = ladd[:, 0:256]
                    nc.tensor.matmul(pl2[:, 0:128], b2, b2l,
                                     start=True, stop=True)        # Nl^4
                    nc.tensor.matmul(pl2[:, 128:256], b2l, b2,
                                     start=True, stop=True)        # Nu^4
                    sb4 = recp.tile([128, 256], bf16, name=f"sb4{hc}",
                                    tag="sb4")
                    nc.scalar.copy(sb4[:], pl2[:])
                    b4l, b4u = sb4[:, 0:128], sb4[:, 128:256]

                    pr8 = ladd[:, 0:256]
                    nc.tensor.matmul(pr8[:, 0:128], b4l, sT1[:],
                                     start=True, stop=False)       # B^4 T1
                    nc.tensor.matmul(pr8[:, 0:128], ident, sT1[:],
                                     start=False, stop=True)
                    nc.tensor.matmul(pr8[:, 128:256], b4u, b4l,
                                     start=True, stop=True)        # Nl^8
                    sr8 = recp.tile([128, 256], bf16, name=f"sr8{hc}",
                                    tag="sr8")
                    nc.scalar.copy(sr8[:], pr8[:])
                    R8, b8l = sr8[:, 0:128], sr8[:, 128:256]

                    prm = ladd[:, 256:384]
                    nc.tensor.matmul(prm[:], b8l, R8, start=True, stop=False)
                    nc.tensor.matmul(prm[:], ident, R8, start=False, stop=True)
                    sRm = recp.tile([128, 128], bf16, name=f"sRm{hc}",
                                    tag="sRm")
                    nc.vector.tensor_copy(sRm[:], prm[:])

                    # full R^T = Rm (I - G^T);  -G^T = F^T-neg @ Rm
                    pn1 = ladd[:, 384:512]
                    nc.tensor.matmul(pn1[:], mFl[:], sRm[:], start=True,
                                     stop=True)
                    prl = ptr[:, 512:640]
                    nc.tensor.transpose(prl[:], sRm[:], ident)
                    sn1 = recp.tile([128, 256], bf16, name=f"sn1{hc}",
                                    tag="sn1")
                    nc.vector.tensor_copy(sn1[:, 0:128], pn1[:])
                    nc.vector.tensor_copy(sn1[:, 128:256], prl[:])
                    n1, Rl = sn1[:, 0:128], sn1[:, 128:256]

                    pRT = ladd[:, 256:384]
                    nc.tensor.matmul(pRT[:], Rl, n1, start=True, stop=False)
                    nc.tensor.matmul(pRT[:], Rl, ident, start=False, stop=True)
                    sRT = recp.tile([128, 128], bf16, name=f"sRT{hc}",
                                    tag="sRT")
                    nc.vector.tensor_copy(sRT[:], pRT[:])

                    # -M^T = -(K^T R^T)  [Dh, C] as two 128-col slices
                    pmt = ladd[:, 0:256]
                    for i in range(2):
                        nc.tensor.matmul(pmt[:, i * 128:(i + 1) * 128],
                                         kcd[:, i * 128:(i + 1) * 128], sRT[:],
                                         start=True, stop=True)
                    smtn = recp.tile([128, 256], bf16, name=f"smtn{hc}",
                                     tag="smtn")
                    nc.vector.tensor_scalar_mul(smtn[:], pmt[:], -1.0)

                    # U' = R V - M S_prev; upo packs [pu | po]
                    upo = psw.tile([128, 512], fp32, name=f"upo{hc}",
                                   tag="upo")
                    pu = upo[:, 0:256]
                    nc.tensor.matmul(pu[:], sRT[:], vcd, start=True,
                                     stop=False)
                    for i in range(2):
                        nc.tensor.matmul(pu[:], smtn[:, i * 128:(i + 1) * 128],
                                         s_sb[head][:, i * 256:(i + 1) * 256],
                                         start=False, stop=(i == 1))
                    u_sb = recp.tile([128, 256], bf16, name=f"u{hc}", tag="u")
                    nc.vector.tensor_copy(u_sb[:], pu[:])

                    # O = P^T U' + Q S_prev; the u-dependent matmul opens
                    # the group so it cannot start while pu is mid-group in
                    # the same bank.
                    po = upo[:, 256:512]
                    nc.tensor.matmul(po[:], sPat[:], u_sb[:], start=True,
                                     stop=False)
                    for i in range(2):
                        nc.tensor.matmul(po[:], QT[i],
                                         s_sb[head][:, i * 256:(i + 1) * 256],
                                         start=False, stop=(i == 1))

                    # S += K^T U'   (persistent psum accumulator)
                    for i in range(2):
                        # start only on the very first write: PE "start"
                        # flags the whole 2KB bank pending-zero, so a second
                        # start would make the next chunk's accumulate into
                        # the other half overwrite instead of add.
                        nc.tensor.matmul(s_ps[head][:, i * 256:(i + 1) * 256],
                                         kcd[:, i * 128:(i + 1) * 128], u_sb[:],
                                         start=(ch == 0 and i == 0), stop=True,
                                         skip_group_check=True)
                    s_nb = recs.tile([128, 512], bf16, name=f"ssb{hc}",
                                     tag=f"ssb{head}", bufs=2)
                    nc.vector.tensor_copy(s_nb[:, 0:256], s_ps[head][:, 0:256])
                    nc.scalar.copy(s_nb[:, 256:512],
                                   s_ps[head][:, 256:512])
                    s_sb[head] = s_nb

                    # RMSNorm rows of O, transpose into oT[ch]
                    osq = recp.tile([128, 256], bf16, name=f"osq{hc}",
                                    tag="osq")
                    ossq = recp.tile([128, 1], fp32, name=f"ossq{hc}",
                                     tag="ossq")
                    nc.scalar.activation(osq[:], po[:], AF.Square,
                                         accum_out=ossq[:])
                    orsq = recp.tile([128, 1], fp32, name=f"orsq{hc}",
                                     tag="orsq")
                    nc.scalar.activation(orsq[:], ossq[:], AF.Sqrt,
                                         bias=biases[:, 1:2], scale=1.0 / DH)
                    nc.vector.reciprocal(orsq[:], orsq[:])
                    onrm = recp.tile([128, 256], bf16, name=f"onrm{hc}",
                                     tag="onrm")
                    nc.vector.tensor_scalar_mul(onrm[:], po[:], orsq[:])
                    pto = ptr[:, 640:896]
                    for i in range(2):
                        nc.tensor.transpose(pto[:, i * 128:(i + 1) * 128],
                                            onrm[:, i * 128:(i + 1) * 128],
                                            ident)
                    nc.scalar.copy(oTt[:, ct0 * 128:ct0 * 128 + 256], pto[:])

                # ---- phase C for token tile ch (overlaps next chunks) ----
                for half in range(2):
                    pf = psw.tile([128, 512], fp32, name=f"pf{ch}{half}",
                                  tag="upo")
                    for ct in range(CT):
                        nc.tensor.matmul(
                            pf[:], oTt[:, ct * 128:(ct + 1) * 128],
                            wo_s[ct][:, half * 512:(half + 1) * 512],
                            start=(ct == 0), stop=(ct == CT - 1))
                    of = ofp.tile([128, 512], fp32, name=f"of{ch}{half}",
                                  tag="of")
                    nc.scalar.copy(of[:], pf[:])
                    nc.sync.dma_start(
                        out_t[ch][:, half * 512:(half + 1) * 512], of[:])


LP_NP = np.float16  # host-side 16-bit dtype matching the device dtype


def _make_consts():
    ii = np.arange(128)
    blk = ii[:, None] // 64 == ii[None, :] // 64
    low = ii[:, None] > ii[None, :]
    nln = -((low & blk).astype(np.float32))
    nun = nln.T.copy()
    fln = -((low & ~blk).astype(np.float32))
    triuI = (ii[:, None] <= ii[None, :]).astype(np.float32)
    ident = np.eye(128, dtype=np.float32)
    return np.concatenate([ident, nln, nun, fln, triuI],
                          axis=1).astype(LP_NP)


def _get_compiled():
    key = ("nc", SILU_NATIVE)
    if key not in _CACHE:
        _CACHE[key] = _build_bass()
    return _CACHE[key]


def kernel(hidden_states, Wq, Wk, Wv, conv_wq, conv_wk, conv_wv, onorm_w, Wo):
    from concourse.bass_utils import run_bass_kernel_spmd

    hidden_states = np.asarray(hidden_states, np.float32)
    Wq = np.asarray(Wq, np.float32)
    Wk = np.asarray(Wk, np.float32)
    Wv = np.asarray(Wv, np.float32)
    Wo = np.asarray(Wo, np.float32)
    conv_wq = np.asarray(conv_wq, np.float32)
    conv_wk = np.asarray(conv_wk, np.float32)
    conv_wv = np.asarray(conv_wv, np.float32)
    onorm_w = np.asarray(onorm_w, np.float32)

    bf = LP_NP
    consts = _make_consts()
    Wo_eff = (Wo * np.tile(onorm_w, H)[:, None]).astype(bf)  # fold RMS weight

    in_maps = []
    for core in range(NCORES):
        b, g = divmod(core, 2)
        cols = slice(CG * g, CG * (g + 1))
        in_maps.append({
            "xT": np.ascontiguousarray(hidden_states[b].T).astype(bf),
            "wq": np.ascontiguousarray(Wq[:, cols]).astype(bf),
            "wk": np.ascontiguousarray(Wk[:, cols]).astype(bf),
            "wv": np.ascontiguousarray(Wv[:, cols]).astype(bf),
            "wo": np.ascontiguousarray(Wo_eff[cols, :]),
            "cw": np.ascontiguousarray(np.concatenate(
                [conv_wq[cols], conv_wk[cols], conv_wv[cols]], axis=1)),
            "consts": consts,
        })

    nc = _get_compiled()
    res = run_bass_kernel_spmd(nc, in_maps, core_ids=list(range(NCORES)),
                               **_CACHE.get("run_kwargs", {}))
    _CACHE["last_results"] = res
    out = np.zeros((B, T, D), np.float32)
    for core in range(NCORES):
        out[core // 2] += res.results[core]["out"]
    return out
